# revision 38
# baseline (speedup 1.0000x reference)
"""NonLocal block (B=4, C=256, H=W=96, D=128) on 8 TRN2 NeuronCores.

Sharding: 8 shards = (sample b = core//2) x (query half qh = core%2).
Each core handles 4608 queries vs all 9216 keys of its sample.

Per-core kernel (projection + attention matmuls in bf16/fp16 with fp32 PSUM
accumulation; scores in fp32, softmax split across ACT and DVE):
  thetaT [D, 4608]  = w_theta @ xq + b_theta          (f16)
  phiT   [D, 9216]  = w_phi   @ xk + b_phi            (f16)
  g      [9216, D]  = xk.T @ w_g.T                    (bf16; bias folded out)
  for each query chunk (512 q) and key-block pair g (2x128 keys):
    ST = phiT_blk.T @ thetaT_chunk          [128 keys, 2*512]  (PE, fp32 PSUM)
    P[:, queries 0:QA]   = exp(ST - 64)     bf16               (ACT spline exp)
    P[:, queries QA:512] = schraudolph(ST)  bf16-bits via u16  (DVE, 1 op)
    Y += g_blk.T @ P_half                   [128 d, 512 q]     (PE, psum accum)
  per oct (4 groups = 8 key blocks), P tiles live in one [128,4096] megatile:
    3-op DVE fold tree (q01+q23 -> oh, bf16 2x), one fold step per group
    L += ones128.T @ oh_half                (PE, two 512-col matmuls per oct,
                                             split across exp windows)
  rl = recip_approx(L); yn = bf16(yT*rl); z = wo.T @ yn + (b_out + w_out@b_g) + xq

Design notes:
 - ACT exp is the bottleneck engine at 1 elem/cycle/lane @1.2GHz: 42.5M
   exps/core => ~280us floor on ACT alone; the kernel runs at ~95% of that.
   A DVE/GPSIMD Schraudolph bit-trick exp (u16 = round(S*128*log2e + B)
   saturating at 0; the u16 bits ARE the bf16 exp; verified bit-exact on
   HW, end-to-end error ~1e-2 vs tolerance 2e-2) can offload a slice of
   the exp columns (BASS_NL_QA < 1024), but measured SLOWER: the ST-psum
   slot WAR chain exp(g) -> ST(g+2) -> exp(g+2) plus tile-granular WAR/WAW
   tracking and strict engine FIFOs serialize the rails; every offloaded
   element returns as ACT idle. Kept for reference, disabled by default.
 - The softmax denominator fold tree runs on [128,4096] P megatiles (4
   groups per tile): 3 tensor_tensor folds + 2 ones-matmuls per oct (vs 7
   [128,512] DVE ops + 1 matmul): fewer per-op overheads and less DVE time
   overall; fold steps are emitted only at phases where their inputs are
   already complete so the DVE FIFO head never blocks.
 - reciprocal_approx_fast (1 DVE op, ~51 ULP) replaces the 6.5 cyc/elem
   iterative reciprocal; yn = bf16(yT*rl) with an early yT copy that frees
   the Y accumulator bank before the fold flush at chunk boundaries.
 - phi's bias-add + f16 cast rides ACT's free affine (Identity, in the exp
   table set) to keep chunk-0 DVE load down; chunk 0 is PE-bound by the
   phi/g projections either way.
 - bf16/f16 moving operands stream through the PE at 1 col/cycle; scores
   are ACCUMULATED in fp32 PSUM (f16 inputs only perturb S by ~0.05 which
   the softmax normalization absorbs).
 - Emission is software-pipelined: each group's ST matmul is emitted BEFORE
   the previous group's Y matmuls (2-deep score lookahead) so the PE always
   has independent work while ACT computes exp; projections for the next
   chunk are hooked into fixed groups, the epilogue tail is deferred into
   the next chunk. A dummy exp at kernel start prefetches the ACT table set
   (~2.7us) off the critical path.

env BASS_NL_REPS=K wraps the whole computation in a K-iteration hardware
loop (idempotent recompute) for slope-based timing. Default 1.
env BASS_NL_QA=n sets the ACT/DVE exp split point (default 1024 = all-ACT;
measured fastest -- see the design note on why offload loses).
Startup DMAs are spread across the SP/ACT/GPSIMD-SWDGE issue queues to
shorten the ramp; the epilogue tail is emitted one channel-half per group.
"""

import os
from contextlib import ExitStack

import ml_dtypes
import numpy as np

import concourse.bass as bass
import concourse.mybir as mybir
import concourse.tile as tile
from concourse import bacc
from concourse.bass_utils import run_bass_kernel_spmd

F32 = mybir.dt.float32
BF16 = mybir.dt.bfloat16
F16 = mybir.dt.float16
U16 = mybir.dt.uint16
AF = mybir.ActivationFunctionType
ALU = mybir.AluOpType

C, N, D = 256, 9216, 128
NQ = N // 2            # queries per core
QCH = 512              # query chunk (one PSUM bank of fp32)
NQC = NQ // QCH        # 9 query chunks
MB = N // 128          # 72 key blocks
NG = MB // 2           # 36 key-block pairs (groups) per query chunk
NOC = NG // 4          # 9 octs per query chunk
NCH = N // 512         # 18 x-chunks (4 key blocks each)
SHIFT = -64.0          # softmax shift constant
LOG2E = 1.4426950408889634
SCH_A = 128.0 * LOG2E                       # Schraudolph scale
SCH_B = 128.0 * 127.0 + SHIFT * SCH_A - 5.6  # bias incl. minimax correction

_CACHE: dict = {}

def _build_nc():
    reps = int(os.environ.get("BASS_NL_REPS", "1"))
    # ACT/DVE exp split point (ACT computes score cols [0:qa), DVE-Schraudolph
    # the rest). 1024 = all-ACT: measured fastest — any offload loses more to
    # cross-engine WAR/WAW serialization of the ST-slot and P-tile than it
    # saves on ACT (tile-granular dep tracking + strict engine FIFOs).
    qa = int(os.environ.get("BASS_NL_QA", "1024"))
    nc = bacc.Bacc("TRN2", target_bir_lowering=False, debug=False, num_devices=8)
    xkb = nc.dram_tensor("xkb", [C, N], F16, kind="ExternalInput").ap()
    xqb = nc.dram_tensor("xqb", [C, NQ], F16, kind="ExternalInput").ap()
    xq = nc.dram_tensor("xq", [C, NQ], F32, kind="ExternalInput").ap()
    wth = nc.dram_tensor("wth", [C, D], F16, kind="ExternalInput").ap()
    wph = nc.dram_tensor("wph", [C, D], F16, kind="ExternalInput").ap()
    wg = nc.dram_tensor("wg", [C, D], F16, kind="ExternalInput").ap()
    wo = nc.dram_tensor("wo", [D, C], BF16, kind="ExternalInput").ap()
    bth = nc.dram_tensor("bth", [D, 1], F32, kind="ExternalInput").ap()
    bph = nc.dram_tensor("bph", [D, 1], F32, kind="ExternalInput").ap()
    bo2 = nc.dram_tensor("bo2", [C, 1], F32, kind="ExternalInput").ap()
    onesd = nc.dram_tensor("onesd", [D, D], BF16, kind="ExternalInput").ap()
    out = nc.dram_tensor("out", [C, NQ], F32, kind="ExternalOutput").ap()

    with tile.TileContext(nc) as tc, ExitStack() as ctx:
        consts = ctx.enter_context(tc.tile_pool(name="consts", bufs=1))
        big = ctx.enter_context(tc.tile_pool(name="big", bufs=1))

        # Persistent SBUF tensors, chunked for block-granular dependencies
        phi_t = [big.tile([128, 512], F16, name=f"phi{i}", tag=f"phi{i}") for i in range(NCH)]
        g_t = [big.tile([128, 512], BF16, name=f"g{i}", tag=f"g{i}") for i in range(NCH)]
        th_t = [big.tile([128, 512], F16, name=f"th{i}", tag=f"th{i}") for i in range(NQC)]

        wth_s = consts.tile([128, 256], F16)
        wph_s = consts.tile([128, 256], F16)
        wg_s = consts.tile([128, 256], F16)
        wo_s = consts.tile([128, 256], BF16)
        bth_s = consts.tile([128, 1], F32)
        bph_s = consts.tile([128, 1], F32)
        bo2_s = consts.tile([128, 2], F32)
        neg_s = consts.tile([128, 1], F32)
        scr_s = consts.tile([128, 1], F32)
        ones_s = consts.tile([128, 128], BF16)

        def body():
            # prefetch the exp table set while the first DMAs are in flight;
            # const DMAs are ordered so the first-phi-chunk critical chain
            # (wph, bph, xk chunk) issues ahead of everything else on the
            # HWDGE FIFO.
            nc.vector.memset(neg_s[:], SHIFT)
            nc.scalar.activation(scr_s[:], neg_s[:], AF.Exp)

            def late_consts():
                nc.scalar.dma_start(out=ones_s[:], in_=onesd[:])
                nc.scalar.dma_start(out=wo_s[:], in_=wo[:])
                nc.scalar.dma_start(out=bo2_s[:, 0:1], in_=bo2[0:128, :])
                nc.scalar.dma_start(out=bo2_s[:, 1:2], in_=bo2[128:256, :])

            with tc.tile_pool(name="psA", bufs=2, space="PSUM") as psA, tc.tile_pool(
                name="xsA", bufs=3
            ) as xsA, tc.tile_pool(name="st", bufs=2, space="PSUM") as stp, tc.tile_pool(
                name="yps", bufs=1, space="PSUM"
            ) as ypp, tc.tile_pool(
                name="lps", bufs=1, space="PSUM"
            ) as lpp, tc.tile_pool(
                name="pexp", bufs=3
            ) as pxp, tc.tile_pool(name="red", bufs=2) as red, tc.tile_pool(
                name="epi", bufs=3
            ) as epi:

                def emit_theta(i, first=False):
                    sl = bass.ts(i, 512)
                    # at startup, issue the theta-path DMAs from the idle
                    # GPSIMD SWDGE queue so they don't serialize behind the
                    # phi chain's DMAs on the SP queue (ramp-critical)
                    eng = nc.gpsimd if first else nc.sync
                    if first:
                        eng.dma_start(out=wth_s[:, 0:128], in_=wth[0:128, :])
                        eng.dma_start(
                            out=wth_s[:, 128:256], in_=wth[128:256, :]
                        )
                        eng.dma_start(out=bth_s[:], in_=bth[:])
                    xq0 = xsA.tile([128, 512], F16, tag="xq0")
                    xq1 = xsA.tile([128, 512], F16, tag="xq1")
                    eng.dma_start(out=xq0[:], in_=xqb[0:128, sl])
                    eng.dma_start(out=xq1[:], in_=xqb[128:256, sl])
                    ps = psA.tile([128, 512], F32, tag="ps")
                    nc.tensor.matmul(
                        ps[:], lhsT=wth_s[:, 0:128], rhs=xq0[:], start=True, stop=False
                    )
                    nc.tensor.matmul(
                        ps[:], lhsT=wth_s[:, 128:256], rhs=xq1[:], start=False, stop=True
                    )
                    nc.vector.tensor_scalar_add(th_t[i][:], ps[:], bth_s[:])

                def emit_phi(i):
                    # phi chunk i and g chunk i share one xk-chunk DMA; the
                    # phi part must precede ST(i*2) in the PE FIFO, the g
                    # part is emitted after it (emit_g) to not delay exp.
                    sl = bass.ts(i, 512)
                    xc0 = xsA.tile([128, 512], F16, tag="xc0")
                    xc1 = xsA.tile([128, 512], F16, tag="xc1")
                    nc.sync.dma_start(out=xc0[:], in_=xkb[0:128, sl])
                    nc.sync.dma_start(out=xc1[:], in_=xkb[128:256, sl])
                    if i == 0:
                        # weight DMAs issued after the ramp-critical first
                        # xk chunk on the SP queue
                        nc.sync.dma_start(out=wph_s[:, 0:128], in_=wph[0:128, :])
                        nc.sync.dma_start(
                            out=wph_s[:, 128:256], in_=wph[128:256, :]
                        )
                        nc.sync.dma_start(out=bph_s[:], in_=bph[:])
                    ps = psA.tile([128, 512], F32, tag="ps")
                    nc.tensor.matmul(
                        ps[:], lhsT=wph_s[:, 0:128], rhs=xc0[:], start=True, stop=False
                    )
                    nc.tensor.matmul(
                        ps[:], lhsT=wph_s[:, 128:256], rhs=xc1[:], start=False, stop=True
                    )
                    # bias-add + f16 cast via ACT's free affine (Identity is
                    # in the exp table set): keeps chunk-0 DVE load down.
                    nc.scalar.activation(
                        phi_t[i][:], ps[:], AF.Identity, bias=bph_s[:]
                    )
                    return xc0, xc1

                def emit_g(i, xc0, xc1, js=(0, 1, 2, 3), pg=None):
                    # each 128-col g matmul is shorter (53ns) than its
                    # LDWEIGHTS (107ns), so bursts of them are LDW-bound;
                    # emitted in two halves interleaved between the 512-col
                    # ST/Y matmuls, whose streams hide the weight loads.
                    if pg is None:
                        pg = psA.tile([128, 512], F32, tag="ps")
                    for j in js:
                        jsl = bass.ts(j, 128)
                        nc.tensor.matmul(
                            pg[:, jsl], lhsT=xc0[:, jsl], rhs=wg_s[:, 0:128],
                            start=True, stop=False,
                        )
                        nc.tensor.matmul(
                            pg[:, jsl], lhsT=xc1[:, jsl], rhs=wg_s[:, 128:256],
                            start=False, stop=True,
                        )
                    if js[-1] == 3:
                        nc.vector.tensor_copy(g_t[i][:], pg[:])
                    return pg

                def emit_st(qc, g):
                    # scores for key blocks 2g, 2g+1 vs query chunk qc.
                    # Block b1's matmul is split at the ACT/DVE boundary ka
                    # so the WAR chain exp(g) -> ST(g+2) -> exp(g+2) only
                    # contains the matmuls the exp actually reads; the
                    # Schraudolph columns [ka:1024) form their own (slack)
                    # DVE rail.
                    ka = 2 * QCH if qc == 0 else qa
                    st = stp.tile([128, 2 * QCH], F32, tag="st")
                    mb0, mb1 = 2 * g, 2 * g + 1
                    lhs0 = phi_t[mb0 // 4][:, bass.ts(mb0 % 4, 128)]
                    lhs1 = phi_t[mb1 // 4][:, bass.ts(mb1 % 4, 128)]
                    nc.tensor.matmul(
                        st[:, 0:QCH], lhsT=lhs0, rhs=th_t[qc][:],
                        start=True, stop=True,
                    )
                    if ka >= 2 * QCH:
                        nc.tensor.matmul(
                            st[:, QCH : 2 * QCH], lhsT=lhs1, rhs=th_t[qc][:],
                            start=True, stop=True,
                        )
                    else:
                        kb = ka - QCH
                        nc.tensor.matmul(
                            st[:, QCH:ka], lhsT=lhs1, rhs=th_t[qc][:, 0:kb],
                            start=True, stop=True,
                        )
                        nc.tensor.matmul(
                            st[:, ka : 2 * QCH], lhsT=lhs1,
                            rhs=th_t[qc][:, kb:QCH], start=True, stop=True,
                        )
                    return st

                def emit_epi_tail(eqsl, yn, xr0, chans=(0, 1)):
                    # z projection + residual + store for a finished query
                    # chunk; deferred into the NEXT chunk (one 128-channel
                    # half per group, so its PE matmuls never overflow one
                    # exp window and stall ACT).
                    for ch in chans:
                        csl = bass.ts(ch, 128)
                        z_ps = psA.tile([128, QCH], F32, tag="ps")
                        nc.tensor.matmul(
                            z_ps[:], lhsT=wo_s[:, csl],
                            rhs=yn[:], start=True, stop=True,
                        )
                        if ch == 0:
                            xr = xr0
                        else:
                            xr = epi.tile([128, QCH], F32, tag="xr1")
                            nc.sync.dma_start(out=xr[:], in_=xq[csl, eqsl])
                        zo = epi.tile([128, QCH], F32, tag=f"zo{ch}")
                        nc.vector.scalar_tensor_tensor(
                            zo[:], z_ps[:], bo2_s[:, ch : ch + 1], xr[:],
                            ALU.add, ALU.add,
                        )
                        nc.sync.dma_start(out=out[csl, eqsl], in_=zo[:])

                def emit_group(nqc, ng):
                    # auxiliary projection work is hooked one chunk AHEAD of
                    # the score tile that first needs it, covering the
                    # xk-DMA + matmul + bias latency of the phi chain
                    xcs = None
                    if nqc == 0 and ng % 2 == 0 and ng // 2 + 1 < NCH:
                        xcs = emit_phi(ng // 2 + 1)
                    st = emit_st(nqc, ng)
                    if xcs is not None:
                        pg = emit_g(ng // 2 + 1, *xcs, js=(0, 1))
                        g_stash.append((ng // 2 + 1, *xcs, (2,), pg))
                        g_stash.append((ng // 2 + 1, *xcs, (3,), pg))
                    return st

                M_of: dict = {}

                def get_M(gi):
                    # megatile holding P for the oct gi//4 (alloc on demand)
                    oct_i = gi // 4
                    if oct_i not in M_of:
                        M_of[oct_i] = pxp.tile(
                            [128, 8 * QCH], BF16, name="M", tag="M"
                        )
                    return M_of[oct_i]

                def emit_schr(st_cur, gi, ka):
                    # DVE Schraudolph bit-trick exp (u16 bits == bf16 exp)
                    # for score columns [ka:1024) -- ONE contiguous run
                    # (strided engine APs pay the fixed cost per run).
                    if ka >= 2 * QCH:
                        return
                    moff = (gi % 4) * 2 * QCH
                    M = get_M(gi)
                    nc.vector.tensor_scalar(
                        M[:, moff + ka : moff + 2 * QCH].bitcast(U16),
                        st_cur[:, ka : 2 * QCH], SCH_A, SCH_B,
                        ALU.mult, ALU.add,
                    )

                def emit_act_exp(st_cur, gi, ka):
                    # ACT spline exp for score columns [0:ka), contiguous
                    moff = (gi % 4) * 2 * QCH
                    M = get_M(gi)
                    nc.scalar.activation(
                        M[:, moff : moff + ka], st_cur[:, 0:ka],
                        AF.Exp, bias=neg_s[:],
                    )

                groups = [(qc, g) for qc in range(NQC) for g in range(NG)]
                pending = []
                g_stash = []
                pending_epi = None
                y_ps = l_ps = None
                f1 = f2 = oh = None
                fold_q = []

                def emit_fold(step, oct_i, cur_l, j):
                    # deferred fold steps for oct_i: q23 (next oct phase 0),
                    # oh + first ones-matmul (phase 1), second ones-matmul
                    # (phase 2). Inputs are complete at emission so the DVE
                    # FIFO never blocks, and the PE sees at most one 512-col
                    # extra matmul per exp window (the pair in one window
                    # overflowed the exp cadence and stalled ACT).
                    nonlocal f2, oh
                    if step == 0:
                        M = M_of[oct_i]
                        f2 = red.tile([128, 2 * QCH], BF16, name="f2", tag="f2")
                        nc.vector.tensor_add(
                            f2[:], M[:, 4 * QCH : 6 * QCH], M[:, 6 * QCH : 8 * QCH]
                        )
                        del M_of[oct_i]
                    elif step == 1:
                        oh = red.tile([128, 2 * QCH], BF16, name="oh", tag="oh")
                        nc.vector.tensor_add(oh[:], f1[:], f2[:])
                        nc.tensor.matmul(
                            cur_l[:], lhsT=ones_s[:], rhs=oh[:, 0:QCH],
                            start=(j == 0), stop=False,
                        )
                    else:
                        nc.tensor.matmul(
                            cur_l[:], lhsT=ones_s[:], rhs=oh[:, QCH : 2 * QCH],
                            start=False, stop=(j == NOC - 1),
                        )
                xcs = emit_phi(0)
                emit_theta(0, first=True)
                pending.append(emit_st(0, 0))
                nc.scalar.dma_start(out=wg_s[:, 0:128], in_=wg[0:128, :])
                nc.scalar.dma_start(out=wg_s[:, 128:256], in_=wg[128:256, :])
                emit_g(0, *xcs)
                xcs = emit_phi(1)
                emit_g(1, *xcs)
                late_consts()
                for idx, (qc, g) in enumerate(groups):
                    gi = qc * NG + g
                    qsl = bass.ts(qc, QCH)
                    if g == 0:
                        y_ps = ypp.tile([128, QCH], F32, tag="y")
                        l_ps = lpp.tile([128, QCH], F32, tag="l")
                    st_cur = pending.pop(0)
                    # chunk 0 is PE-bound by the phi/g projections: keep all
                    # exp on ACT there; later chunks offload [ka:1024) to DVE
                    ka = 2 * QCH if qc == 0 else qa
                    emit_act_exp(st_cur, gi, ka)
                    # Schraudolph inline: its ST input is 2 groups old so it
                    # never blocks the DVE FIFO head; it runs before any fold
                    # of this iteration, keeping the ST-slot WAR ACT-gated.
                    emit_schr(st_cur, gi, ka)
                    # persistent 2-deep score lookahead: the next TWO groups'
                    # ST matmuls always sit in the PE FIFO ahead of this
                    # group's Y matmuls, so nothing that waits on the DVE
                    # can ever starve ACT of its next exp input.
                    while len(pending) < 2 and idx + 1 + len(pending) < len(groups):
                        nqc, ng = groups[idx + 1 + len(pending)]
                        pending.append(emit_group(nqc, ng))
                    if pending_epi is not None and g in (2, 3):
                        emit_epi_tail(*pending_epi, chans=(g - 2,))
                        if g == 3:
                            pending_epi = None
                    M = get_M(gi)
                    moff = (g % 4) * 2 * QCH
                    for h in range(2):
                        mb = 2 * g + h
                        nc.tensor.matmul(
                            y_ps[:],
                            lhsT=g_t[mb // 4][:, bass.ts(mb % 4, 128)],
                            rhs=M[:, moff + h * QCH : moff + (h + 1) * QCH],
                            start=(mb == 0), stop=(mb == MB - 1),
                        )
                    if g_stash:
                        s = g_stash.pop(0)
                        emit_g(*s[:3], js=s[3], pg=s[4])
                    if qc < NQC - 1 and g == 20:
                        # theta for the NEXT query chunk, emitted after this
                        # group's Y matmuls so its PE work never sits between
                        # an exp and the ST pair that exp's successor needs
                        emit_theta(qc + 1)
                    # denominator folds: q01 inline at phase 2 (inputs are
                    # the phase-0/1 exps, already done); q23 and the oct
                    # finish deferred into the next oct's phases 0/1.
                    if fold_q:
                        emit_fold(*fold_q.pop(0))
                    if g % 4 == 2:
                        f1 = red.tile([128, 2 * QCH], BF16, name="f1", tag="f1")
                        nc.vector.tensor_add(
                            f1[:], M[:, 0 : 2 * QCH], M[:, 2 * QCH : 4 * QCH]
                        )
                    elif g % 4 == 3:
                        for step in range(3):
                            fold_q.append((step, gi // 4, l_ps, g // 4))
                    if g == NG - 1:
                        # free the Y accumulator bank first (bf16 copy), then
                        # flush the remaining folds, 1/L, and the normalized
                        # yn tile. The z/store tail is deferred into the next
                        # chunk. On the final chunk nothing reuses the Y bank,
                        # so normalize straight out of PSUM (shorter tail).
                        last = qc == NQC - 1
                        if not last:
                            yT = epi.tile([128, QCH], BF16, tag="yT")
                            nc.vector.tensor_copy(yT[:], y_ps[:])
                        while fold_q:
                            emit_fold(*fold_q.pop(0))
                        rl = epi.tile([128, QCH], F32, tag="rl")
                        nc.vector.reciprocal_approx_fast(rl[:], l_ps[:])
                        yn = epi.tile([128, QCH], BF16, tag="yn")
                        nc.vector.tensor_mul(
                            yn[:], y_ps[:] if last else yT[:], rl[:]
                        )
                        xr0 = epi.tile([128, QCH], F32, tag="xr0")
                        nc.sync.dma_start(out=xr0[:], in_=xq[0:128, qsl])
                        pending_epi = (qsl, yn, xr0)
                emit_epi_tail(*pending_epi)

        if reps > 1:
            with tc.For_i(0, reps, 1):
                body()
        else:
            body()

    nc.compile()
    return nc


def _get_nc():
    if "nc" not in _CACHE:
        _CACHE["nc"] = _build_nc()
    return _CACHE["nc"]


def _in_maps(x, w_theta, b_theta, w_phi, b_phi, w_g, b_g, w_out, b_out):
    bf = ml_dtypes.bfloat16
    f16 = np.float16
    bo2 = (b_out + w_out @ b_g).astype(np.float32).reshape(C, 1)
    shared = {
        "onesd": np.ones((D, D), bf),
        "wth": np.ascontiguousarray(w_theta.T).astype(f16),
        "wph": np.ascontiguousarray(w_phi.T).astype(f16),
        "wg": np.ascontiguousarray(w_g.T).astype(f16),
        "wo": np.ascontiguousarray(w_out.T).astype(bf),
        "bth": b_theta.reshape(D, 1).copy(),
        "bph": b_phi.reshape(D, 1).copy(),
        "bo2": bo2,
    }
    in_maps = []
    for core in range(8):
        b, qh = core // 2, core % 2
        xkc = np.ascontiguousarray(x[b].reshape(C, N))
        xqc = np.ascontiguousarray(xkc[:, qh * NQ : (qh + 1) * NQ])
        in_maps.append(
            {
                "xkb": xkc.astype(f16),
                "xqb": xqc.astype(f16),
                "xq": xqc,
                **shared,
            }
        )
    return in_maps


def kernel(x, w_theta, b_theta, w_phi, b_phi, w_g, b_g, w_out, b_out, **kw):
    x = np.asarray(x, np.float32)
    w_theta = np.asarray(w_theta, np.float32)
    b_theta = np.asarray(b_theta, np.float32)
    w_phi = np.asarray(w_phi, np.float32)
    b_phi = np.asarray(b_phi, np.float32)
    w_g = np.asarray(w_g, np.float32)
    b_g = np.asarray(b_g, np.float32)
    w_out = np.asarray(w_out, np.float32)
    b_out = np.asarray(b_out, np.float32)

    B = x.shape[0]
    nc = _get_nc()
    in_maps = _in_maps(
        x, w_theta, b_theta, w_phi, b_phi, w_g, b_g, w_out, b_out
    )

    res = run_bass_kernel_spmd(nc, in_maps, list(range(8)))
    z = np.empty((B, C, N), np.float32)
    for core in range(8):
        b, qh = core // 2, core % 2
        z[b][:, qh * NQ : (qh + 1) * NQ] = res.results[core]["out"]
    return z.reshape(x.shape)


# revision 39
# speedup vs baseline: 1.0046x; 1.0046x over previous
"""NonLocal block (B=4, C=256, H=W=96, D=128) on 8 TRN2 NeuronCores.

Sharding: 8 shards = (sample b = core//2) x (query half qh = core%2).
Each core handles 4608 queries vs all 9216 keys of its sample.

Per-core kernel (projection + attention matmuls in bf16/fp16 with fp32 PSUM
accumulation; scores in fp32, softmax split across ACT and DVE):
  thetaT [D, 4608]  = w_theta @ xq + b_theta          (f16)
  phiT   [D, 9216]  = w_phi   @ xk + b_phi            (f16)
  g      [9216, D]  = xk.T @ w_g.T                    (bf16; bias folded out)
  for each query chunk (512 q) and key-block pair g (2x128 keys):
    ST = phiT_blk.T @ thetaT_chunk          [128 keys, 2*512]  (PE, fp32 PSUM)
    P[:, queries 0:QA]   = exp(ST - 64)     bf16               (ACT spline exp)
    P[:, queries QA:512] = schraudolph(ST)  bf16-bits via u16  (DVE, 1 op)
    Y += g_blk.T @ P_half                   [128 d, 512 q]     (PE, psum accum)
  per oct (4 groups = 8 key blocks), P tiles live in one [128,4096] megatile:
    3-op DVE fold tree (q01+q23 -> oh, bf16 2x), one fold step per group
    L += ones128.T @ oh_half                (PE, two 512-col matmuls per oct,
                                             split across exp windows)
  rl = recip_approx(L); yn = bf16(yT*rl); z = wo.T @ yn + (b_out + w_out@b_g) + xq

Design notes:
 - ACT exp is the bottleneck engine at 1 elem/cycle/lane @1.2GHz: 42.5M
   exps/core => ~280us floor on ACT alone; the kernel runs at ~95% of that.
   A DVE/GPSIMD Schraudolph bit-trick exp (u16 = round(S*128*log2e + B)
   saturating at 0; the u16 bits ARE the bf16 exp; verified bit-exact on
   HW, end-to-end error ~1e-2 vs tolerance 2e-2) can offload a slice of
   the exp columns (BASS_NL_QA < 1024), but measured SLOWER: the ST-psum
   slot WAR chain exp(g) -> ST(g+2) -> exp(g+2) plus tile-granular WAR/WAW
   tracking and strict engine FIFOs serialize the rails; every offloaded
   element returns as ACT idle. Kept for reference, disabled by default.
 - The softmax denominator fold tree runs on [128,4096] P megatiles (4
   groups per tile): 3 tensor_tensor folds + 2 ones-matmuls per oct (vs 7
   [128,512] DVE ops + 1 matmul): fewer per-op overheads and less DVE time
   overall; fold steps are emitted only at phases where their inputs are
   already complete so the DVE FIFO head never blocks.
 - reciprocal_approx_fast (1 DVE op, ~51 ULP) replaces the 6.5 cyc/elem
   iterative reciprocal; yn = bf16(yT*rl) with an early yT copy that frees
   the Y accumulator bank before the fold flush at chunk boundaries.
 - phi's bias-add + f16 cast rides ACT's free affine (Identity, in the exp
   table set) to keep chunk-0 DVE load down; chunk 0 is PE-bound by the
   phi/g projections either way.
 - bf16/f16 moving operands stream through the PE at 1 col/cycle; scores
   are ACCUMULATED in fp32 PSUM (f16 inputs only perturb S by ~0.05 which
   the softmax normalization absorbs).
 - Emission is software-pipelined: each group's ST matmul is emitted BEFORE
   the previous group's Y matmuls (2-deep score lookahead) so the PE always
   has independent work while ACT computes exp; projections for the next
   chunk are hooked into fixed groups, the epilogue tail is deferred into
   the next chunk. A dummy exp at kernel start prefetches the ACT table set
   (~2.7us) off the critical path.

env BASS_NL_REPS=K wraps the whole computation in a K-iteration hardware
loop (idempotent recompute) for slope-based timing. Default 1.
env BASS_NL_QA=n sets the ACT/DVE exp split point (default 1024 = all-ACT;
measured fastest -- see the design note on why offload loses).
Startup DMAs are spread across the SP/ACT/GPSIMD-SWDGE issue queues to
shorten the ramp; the epilogue tail is emitted one channel-half per group.
"""

import os
from contextlib import ExitStack

import ml_dtypes
import numpy as np

import concourse.bass as bass
import concourse.mybir as mybir
import concourse.tile as tile
from concourse import bacc
from concourse.bass_utils import run_bass_kernel_spmd

F32 = mybir.dt.float32
BF16 = mybir.dt.bfloat16
F16 = mybir.dt.float16
U16 = mybir.dt.uint16
AF = mybir.ActivationFunctionType
ALU = mybir.AluOpType

C, N, D = 256, 9216, 128
NQ = N // 2            # queries per core
QCH = 512              # query chunk (one PSUM bank of fp32)
NQC = NQ // QCH        # 9 query chunks
MB = N // 128          # 72 key blocks
NG = MB // 2           # 36 key-block pairs (groups) per query chunk
NOC = NG // 4          # 9 octs per query chunk
NCH = N // 512         # 18 x-chunks (4 key blocks each)
SHIFT = -64.0          # softmax shift constant
LOG2E = 1.4426950408889634
SCH_A = 128.0 * LOG2E                       # Schraudolph scale
SCH_B = 128.0 * 127.0 + SHIFT * SCH_A - 5.6  # bias incl. minimax correction

_CACHE: dict = {}

def _build_nc():
    reps = int(os.environ.get("BASS_NL_REPS", "1"))
    # ACT/DVE exp split point (ACT computes score cols [0:qa), DVE-Schraudolph
    # the rest). 1024 = all-ACT: measured fastest — any offload loses more to
    # cross-engine WAR/WAW serialization of the ST-slot and P-tile than it
    # saves on ACT (tile-granular dep tracking + strict engine FIFOs).
    qa = int(os.environ.get("BASS_NL_QA", "1024"))
    nc = bacc.Bacc("TRN2", target_bir_lowering=False, debug=False, num_devices=8)
    xkb = nc.dram_tensor("xkb", [C, N], F16, kind="ExternalInput").ap()
    xqb = nc.dram_tensor("xqb", [C, NQ], F16, kind="ExternalInput").ap()
    xq = nc.dram_tensor("xq", [C, NQ], F32, kind="ExternalInput").ap()
    wth = nc.dram_tensor("wth", [C, D], F16, kind="ExternalInput").ap()
    wph = nc.dram_tensor("wph", [C, D], F16, kind="ExternalInput").ap()
    wg = nc.dram_tensor("wg", [C, D], F16, kind="ExternalInput").ap()
    wo = nc.dram_tensor("wo", [D, C], BF16, kind="ExternalInput").ap()
    bth = nc.dram_tensor("bth", [D, 1], F32, kind="ExternalInput").ap()
    bph = nc.dram_tensor("bph", [D, 1], F32, kind="ExternalInput").ap()
    bo2 = nc.dram_tensor("bo2", [C, 1], F32, kind="ExternalInput").ap()
    onesd = nc.dram_tensor("onesd", [D, D], BF16, kind="ExternalInput").ap()
    out = nc.dram_tensor("out", [C, NQ], F32, kind="ExternalOutput").ap()

    with tile.TileContext(nc) as tc, ExitStack() as ctx:
        consts = ctx.enter_context(tc.tile_pool(name="consts", bufs=1))
        big = ctx.enter_context(tc.tile_pool(name="big", bufs=1))

        # Persistent SBUF tensors, chunked for block-granular dependencies
        phi_t = [big.tile([128, 512], F16, name=f"phi{i}", tag=f"phi{i}") for i in range(NCH)]
        g_t = [big.tile([128, 512], BF16, name=f"g{i}", tag=f"g{i}") for i in range(NCH)]
        th_t = [big.tile([128, 512], F16, name=f"th{i}", tag=f"th{i}") for i in range(NQC)]

        wth_s = consts.tile([128, 256], F16)
        wph_s = consts.tile([128, 256], F16)
        wg_s = consts.tile([128, 256], F16)
        wo_s = consts.tile([128, 256], BF16)
        bth_s = consts.tile([128, 1], F32)
        bph_s = consts.tile([128, 1], F32)
        bo2_s = consts.tile([128, 2], F32)
        neg_s = consts.tile([128, 1], F32)
        scr_s = consts.tile([128, 1], F32)
        ones_s = consts.tile([128, 128], BF16)

        def body():
            # prefetch the exp table set while the first DMAs are in flight;
            # const DMAs are ordered so the first-phi-chunk critical chain
            # (wph, bph, xk chunk) issues ahead of everything else on the
            # HWDGE FIFO.
            nc.vector.memset(neg_s[:], SHIFT)
            nc.scalar.activation(scr_s[:], neg_s[:], AF.Exp)

            def late_consts():
                nc.scalar.dma_start(out=ones_s[:], in_=onesd[:])
                nc.scalar.dma_start(out=wo_s[:], in_=wo[:])
                nc.scalar.dma_start(out=bo2_s[:, 0:1], in_=bo2[0:128, :])
                nc.scalar.dma_start(out=bo2_s[:, 1:2], in_=bo2[128:256, :])

            with tc.tile_pool(name="psA", bufs=2, space="PSUM") as psA, tc.tile_pool(
                name="xsA", bufs=3
            ) as xsA, tc.tile_pool(name="st", bufs=2, space="PSUM") as stp, tc.tile_pool(
                name="yps", bufs=1, space="PSUM"
            ) as ypp, tc.tile_pool(
                name="lps", bufs=1, space="PSUM"
            ) as lpp, tc.tile_pool(
                name="pexp", bufs=3
            ) as pxp, tc.tile_pool(name="red", bufs=2) as red, tc.tile_pool(
                name="epi", bufs=3
            ) as epi:

                def emit_theta(i, first=False):
                    sl = bass.ts(i, 512)
                    # at startup, issue the theta-path DMAs from the idle
                    # GPSIMD SWDGE queue so they don't serialize behind the
                    # phi chain's DMAs on the SP queue (ramp-critical)
                    eng = nc.gpsimd if first else nc.sync
                    if first:
                        eng.dma_start(out=wth_s[:, 0:128], in_=wth[0:128, :])
                        eng.dma_start(
                            out=wth_s[:, 128:256], in_=wth[128:256, :]
                        )
                        eng.dma_start(out=bth_s[:], in_=bth[:])
                    xq0 = xsA.tile([128, 512], F16, tag="xq0")
                    xq1 = xsA.tile([128, 512], F16, tag="xq1")
                    eng.dma_start(out=xq0[:], in_=xqb[0:128, sl])
                    eng.dma_start(out=xq1[:], in_=xqb[128:256, sl])
                    ps = psA.tile([128, 512], F32, tag="ps")
                    nc.tensor.matmul(
                        ps[:], lhsT=wth_s[:, 0:128], rhs=xq0[:], start=True, stop=False
                    )
                    nc.tensor.matmul(
                        ps[:], lhsT=wth_s[:, 128:256], rhs=xq1[:], start=False, stop=True
                    )
                    nc.vector.tensor_scalar_add(th_t[i][:], ps[:], bth_s[:])

                def emit_phi(i):
                    # phi chunk i and g chunk i share one xk-chunk DMA; the
                    # phi part must precede ST(i*2) in the PE FIFO, the g
                    # part is emitted after it (emit_g) to not delay exp.
                    sl = bass.ts(i, 512)
                    xc0 = xsA.tile([128, 512], F16, tag="xc0")
                    xc1 = xsA.tile([128, 512], F16, tag="xc1")
                    nc.sync.dma_start(out=xc0[:], in_=xkb[0:128, sl])
                    nc.sync.dma_start(out=xc1[:], in_=xkb[128:256, sl])
                    if i == 0:
                        # weight DMAs issued after the ramp-critical first
                        # xk chunk on the SP queue
                        nc.sync.dma_start(out=wph_s[:, 0:128], in_=wph[0:128, :])
                        nc.sync.dma_start(
                            out=wph_s[:, 128:256], in_=wph[128:256, :]
                        )
                        nc.sync.dma_start(out=bph_s[:], in_=bph[:])
                    ps = psA.tile([128, 512], F32, tag="ps")
                    nc.tensor.matmul(
                        ps[:], lhsT=wph_s[:, 0:128], rhs=xc0[:], start=True, stop=False
                    )
                    nc.tensor.matmul(
                        ps[:], lhsT=wph_s[:, 128:256], rhs=xc1[:], start=False, stop=True
                    )
                    # bias-add + f16 cast via ACT's free affine (Identity is
                    # in the exp table set): keeps chunk-0 DVE load down.
                    nc.scalar.activation(
                        phi_t[i][:], ps[:], AF.Identity, bias=bph_s[:]
                    )
                    return xc0, xc1

                def emit_g(i, xc0, xc1, js=(0, 1, 2, 3), pg=None):
                    # each 128-col g matmul is shorter (53ns) than its
                    # LDWEIGHTS (107ns), so bursts of them are LDW-bound;
                    # emitted in two halves interleaved between the 512-col
                    # ST/Y matmuls, whose streams hide the weight loads.
                    if pg is None:
                        pg = psA.tile([128, 512], F32, tag="ps")
                    for j in js:
                        jsl = bass.ts(j, 128)
                        nc.tensor.matmul(
                            pg[:, jsl], lhsT=xc0[:, jsl], rhs=wg_s[:, 0:128],
                            start=True, stop=False,
                        )
                        nc.tensor.matmul(
                            pg[:, jsl], lhsT=xc1[:, jsl], rhs=wg_s[:, 128:256],
                            start=False, stop=True,
                        )
                    if js[-1] == 3:
                        nc.vector.tensor_copy(g_t[i][:], pg[:])
                    return pg

                def emit_st(qc, g):
                    # scores for key blocks 2g, 2g+1 vs query chunk qc.
                    # Block b1's matmul is split at the ACT/DVE boundary ka
                    # so the WAR chain exp(g) -> ST(g+2) -> exp(g+2) only
                    # contains the matmuls the exp actually reads; the
                    # Schraudolph columns [ka:1024) form their own (slack)
                    # DVE rail.
                    ka = 2 * QCH if qc == 0 else qa
                    st = stp.tile([128, 2 * QCH], F32, tag="st")
                    mb0, mb1 = 2 * g, 2 * g + 1
                    lhs0 = phi_t[mb0 // 4][:, bass.ts(mb0 % 4, 128)]
                    lhs1 = phi_t[mb1 // 4][:, bass.ts(mb1 % 4, 128)]
                    nc.tensor.matmul(
                        st[:, 0:QCH], lhsT=lhs0, rhs=th_t[qc][:],
                        start=True, stop=True,
                    )
                    if ka >= 2 * QCH:
                        nc.tensor.matmul(
                            st[:, QCH : 2 * QCH], lhsT=lhs1, rhs=th_t[qc][:],
                            start=True, stop=True,
                        )
                    else:
                        kb = ka - QCH
                        nc.tensor.matmul(
                            st[:, QCH:ka], lhsT=lhs1, rhs=th_t[qc][:, 0:kb],
                            start=True, stop=True,
                        )
                        nc.tensor.matmul(
                            st[:, ka : 2 * QCH], lhsT=lhs1,
                            rhs=th_t[qc][:, kb:QCH], start=True, stop=True,
                        )
                    return st

                def emit_epi_tail(eqsl, yn, xr0, chans=(0, 1)):
                    # z projection + residual + store for a finished query
                    # chunk; deferred into the NEXT chunk (one 128-channel
                    # half per group, so its PE matmuls never overflow one
                    # exp window and stall ACT).
                    for ch in chans:
                        csl = bass.ts(ch, 128)
                        z_ps = psA.tile([128, QCH], F32, tag="ps")
                        nc.tensor.matmul(
                            z_ps[:], lhsT=wo_s[:, csl],
                            rhs=yn[:], start=True, stop=True,
                        )
                        if ch == 0:
                            xr = xr0
                        else:
                            xr = epi.tile([128, QCH], F32, tag="xr1")
                            nc.sync.dma_start(out=xr[:], in_=xq[csl, eqsl])
                        zo = epi.tile([128, QCH], F32, tag=f"zo{ch}")
                        nc.vector.scalar_tensor_tensor(
                            zo[:], z_ps[:], bo2_s[:, ch : ch + 1], xr[:],
                            ALU.add, ALU.add,
                        )
                        nc.sync.dma_start(out=out[csl, eqsl], in_=zo[:])

                def emit_group(nqc, ng):
                    # auxiliary projection work is hooked one chunk AHEAD of
                    # the score tile that first needs it, covering the
                    # xk-DMA + matmul + bias latency of the phi chain
                    xcs = None
                    if nqc == 0 and ng % 2 == 0 and ng // 2 + 1 < NCH:
                        xcs = emit_phi(ng // 2 + 1)
                    st = emit_st(nqc, ng)
                    if xcs is not None:
                        pg = emit_g(ng // 2 + 1, *xcs, js=(0, 1))
                        g_stash.append((ng // 2 + 1, *xcs, (2, 3), pg))
                    return st

                M_of: dict = {}

                def get_M(gi):
                    # megatile holding P for the oct gi//4 (alloc on demand)
                    oct_i = gi // 4
                    if oct_i not in M_of:
                        M_of[oct_i] = pxp.tile(
                            [128, 8 * QCH], BF16, name="M", tag="M"
                        )
                    return M_of[oct_i]

                def emit_schr(st_cur, gi, ka):
                    # DVE Schraudolph bit-trick exp (u16 bits == bf16 exp)
                    # for score columns [ka:1024) -- ONE contiguous run
                    # (strided engine APs pay the fixed cost per run).
                    if ka >= 2 * QCH:
                        return
                    moff = (gi % 4) * 2 * QCH
                    M = get_M(gi)
                    nc.vector.tensor_scalar(
                        M[:, moff + ka : moff + 2 * QCH].bitcast(U16),
                        st_cur[:, ka : 2 * QCH], SCH_A, SCH_B,
                        ALU.mult, ALU.add,
                    )

                def emit_act_exp(st_cur, gi, ka):
                    # ACT spline exp for score columns [0:ka), contiguous
                    moff = (gi % 4) * 2 * QCH
                    M = get_M(gi)
                    nc.scalar.activation(
                        M[:, moff : moff + ka], st_cur[:, 0:ka],
                        AF.Exp, bias=neg_s[:],
                    )

                groups = [(qc, g) for qc in range(NQC) for g in range(NG)]
                pending = []
                g_stash = []
                pending_epi = None
                y_ps = l_ps = None
                f1 = f2 = oh = None
                fold_q = []

                def emit_fold(step, oct_i, cur_l, j):
                    # deferred fold steps for oct_i: q23 (next oct phase 0),
                    # oh + first ones-matmul (phase 1), second ones-matmul
                    # (phase 2). Inputs are complete at emission so the DVE
                    # FIFO never blocks, and the PE sees at most one 512-col
                    # extra matmul per exp window (the pair in one window
                    # overflowed the exp cadence and stalled ACT).
                    nonlocal f2, oh
                    if step == 0:
                        M = M_of[oct_i]
                        f2 = red.tile([128, 2 * QCH], BF16, name="f2", tag="f2")
                        nc.vector.tensor_add(
                            f2[:], M[:, 4 * QCH : 6 * QCH], M[:, 6 * QCH : 8 * QCH]
                        )
                        del M_of[oct_i]
                    elif step == 1:
                        oh = red.tile([128, 2 * QCH], BF16, name="oh", tag="oh")
                        nc.vector.tensor_add(oh[:], f1[:], f2[:])
                        nc.tensor.matmul(
                            cur_l[:], lhsT=ones_s[:], rhs=oh[:, 0:QCH],
                            start=(j == 0), stop=False,
                        )
                    else:
                        nc.tensor.matmul(
                            cur_l[:], lhsT=ones_s[:], rhs=oh[:, QCH : 2 * QCH],
                            start=False, stop=(j == NOC - 1),
                        )
                xcs = emit_phi(0)
                emit_theta(0, first=True)
                pending.append(emit_st(0, 0))
                nc.scalar.dma_start(out=wg_s[:, 0:128], in_=wg[0:128, :])
                nc.scalar.dma_start(out=wg_s[:, 128:256], in_=wg[128:256, :])
                emit_g(0, *xcs)
                xcs = emit_phi(1)
                emit_g(1, *xcs)
                late_consts()
                for idx, (qc, g) in enumerate(groups):
                    gi = qc * NG + g
                    qsl = bass.ts(qc, QCH)
                    if g == 0:
                        y_ps = ypp.tile([128, QCH], F32, tag="y")
                        l_ps = lpp.tile([128, QCH], F32, tag="l")
                    st_cur = pending.pop(0)
                    # chunk 0 is PE-bound by the phi/g projections: keep all
                    # exp on ACT there; later chunks offload [ka:1024) to DVE
                    ka = 2 * QCH if qc == 0 else qa
                    emit_act_exp(st_cur, gi, ka)
                    # Schraudolph inline: its ST input is 2 groups old so it
                    # never blocks the DVE FIFO head; it runs before any fold
                    # of this iteration, keeping the ST-slot WAR ACT-gated.
                    emit_schr(st_cur, gi, ka)
                    # persistent 2-deep score lookahead: the next TWO groups'
                    # ST matmuls always sit in the PE FIFO ahead of this
                    # group's Y matmuls, so nothing that waits on the DVE
                    # can ever starve ACT of its next exp input.
                    while len(pending) < 2 and idx + 1 + len(pending) < len(groups):
                        nqc, ng = groups[idx + 1 + len(pending)]
                        pending.append(emit_group(nqc, ng))
                    if pending_epi is not None and g in (2, 3):
                        emit_epi_tail(*pending_epi, chans=(g - 2,))
                        if g == 3:
                            pending_epi = None
                    M = get_M(gi)
                    moff = (g % 4) * 2 * QCH
                    for h in range(2):
                        mb = 2 * g + h
                        nc.tensor.matmul(
                            y_ps[:],
                            lhsT=g_t[mb // 4][:, bass.ts(mb % 4, 128)],
                            rhs=M[:, moff + h * QCH : moff + (h + 1) * QCH],
                            start=(mb == 0), stop=(mb == MB - 1),
                        )
                    if g_stash:
                        s = g_stash.pop(0)
                        emit_g(*s[:3], js=s[3], pg=s[4])
                    if qc < NQC - 1 and g == 20:
                        # theta for the NEXT query chunk, emitted after this
                        # group's Y matmuls so its PE work never sits between
                        # an exp and the ST pair that exp's successor needs
                        emit_theta(qc + 1)
                    # denominator folds: q01 inline at phase 2 (inputs are
                    # the phase-0/1 exps, already done); q23 and the oct
                    # finish deferred into the next oct's phases 0/1.
                    if fold_q:
                        emit_fold(*fold_q.pop(0))
                    if g % 4 == 2:
                        f1 = red.tile([128, 2 * QCH], BF16, name="f1", tag="f1")
                        nc.vector.tensor_add(
                            f1[:], M[:, 0 : 2 * QCH], M[:, 2 * QCH : 4 * QCH]
                        )
                    elif g % 4 == 3:
                        for step in range(3):
                            fold_q.append((step, gi // 4, l_ps, g // 4))
                    if g == NG - 1:
                        # free the Y accumulator bank first (bf16 copy), then
                        # flush the remaining folds, 1/L, and the normalized
                        # yn tile. The z/store tail is deferred into the next
                        # chunk. On the final chunk nothing reuses the Y bank,
                        # so normalize straight out of PSUM (shorter tail).
                        last = qc == NQC - 1
                        if not last:
                            yT = epi.tile([128, QCH], BF16, tag="yT")
                            nc.vector.tensor_copy(yT[:], y_ps[:])
                        while fold_q:
                            emit_fold(*fold_q.pop(0))
                        rl = epi.tile([128, QCH], F32, tag="rl")
                        nc.vector.reciprocal_approx_fast(rl[:], l_ps[:])
                        yn = epi.tile([128, QCH], BF16, tag="yn")
                        nc.vector.tensor_mul(
                            yn[:], y_ps[:] if last else yT[:], rl[:]
                        )
                        xr0 = epi.tile([128, QCH], F32, tag="xr0")
                        nc.sync.dma_start(out=xr0[:], in_=xq[0:128, qsl])
                        pending_epi = (qsl, yn, xr0)
                emit_epi_tail(*pending_epi)

        if reps > 1:
            with tc.For_i(0, reps, 1):
                body()
        else:
            body()

    nc.compile()
    return nc


def _get_nc():
    if "nc" not in _CACHE:
        _CACHE["nc"] = _build_nc()
    return _CACHE["nc"]


def _in_maps(x, w_theta, b_theta, w_phi, b_phi, w_g, b_g, w_out, b_out):
    bf = ml_dtypes.bfloat16
    f16 = np.float16
    bo2 = (b_out + w_out @ b_g).astype(np.float32).reshape(C, 1)
    shared = {
        "onesd": np.ones((D, D), bf),
        "wth": np.ascontiguousarray(w_theta.T).astype(f16),
        "wph": np.ascontiguousarray(w_phi.T).astype(f16),
        "wg": np.ascontiguousarray(w_g.T).astype(f16),
        "wo": np.ascontiguousarray(w_out.T).astype(bf),
        "bth": b_theta.reshape(D, 1).copy(),
        "bph": b_phi.reshape(D, 1).copy(),
        "bo2": bo2,
    }
    in_maps = []
    for core in range(8):
        b, qh = core // 2, core % 2
        xkc = np.ascontiguousarray(x[b].reshape(C, N))
        xqc = np.ascontiguousarray(xkc[:, qh * NQ : (qh + 1) * NQ])
        in_maps.append(
            {
                "xkb": xkc.astype(f16),
                "xqb": xqc.astype(f16),
                "xq": xqc,
                **shared,
            }
        )
    return in_maps


def kernel(x, w_theta, b_theta, w_phi, b_phi, w_g, b_g, w_out, b_out, **kw):
    x = np.asarray(x, np.float32)
    w_theta = np.asarray(w_theta, np.float32)
    b_theta = np.asarray(b_theta, np.float32)
    w_phi = np.asarray(w_phi, np.float32)
    b_phi = np.asarray(b_phi, np.float32)
    w_g = np.asarray(w_g, np.float32)
    b_g = np.asarray(b_g, np.float32)
    w_out = np.asarray(w_out, np.float32)
    b_out = np.asarray(b_out, np.float32)

    B = x.shape[0]
    nc = _get_nc()
    in_maps = _in_maps(
        x, w_theta, b_theta, w_phi, b_phi, w_g, b_g, w_out, b_out
    )

    res = run_bass_kernel_spmd(nc, in_maps, list(range(8)))
    z = np.empty((B, C, N), np.float32)
    for core in range(8):
        b, qh = core // 2, core % 2
        z[b][:, qh * NQ : (qh + 1) * NQ] = res.results[core]["out"]
    return z.reshape(x.shape)


# revision 40
# speedup vs baseline: 1.0145x; 1.0098x over previous
"""NonLocal block (B=4, C=256, H=W=96, D=128) on 8 TRN2 NeuronCores.

Sharding: 8 shards = (sample b = core//2) x (query half qh = core%2).
Each core handles 4608 queries vs all 9216 keys of its sample.

Per-core kernel (projection + attention matmuls in bf16/fp16 with fp32 PSUM
accumulation; scores in fp32, softmax split across ACT and DVE):
  thetaT [D, 4608]  = w_theta @ xq + b_theta          (f16)
  phiT   [D, 9216]  = w_phi   @ xk + b_phi            (f16)
  g      [9216, D]  = xk.T @ w_g.T                    (bf16; bias folded out)
  for each query chunk (512 q) and key-block pair g (2x128 keys):
    ST = phiT_blk.T @ thetaT_chunk          [128 keys, 2*512]  (PE, fp32 PSUM)
    P[:, queries 0:QA]   = exp(ST - 64)     bf16               (ACT spline exp)
    P[:, queries QA:512] = schraudolph(ST)  bf16-bits via u16  (DVE, 1 op)
    Y += g_blk.T @ P_half                   [128 d, 512 q]     (PE, psum accum)
  per oct (4 groups = 8 key blocks), P tiles live in one [128,4096] megatile:
    3-op DVE fold tree (q01+q23 -> oh, bf16 2x), one fold step per group
    L += ones128.T @ oh_half                (PE, two 512-col matmuls per oct,
                                             split across exp windows)
  rl = recip_approx(L); yn = bf16(yT*rl); z = wo.T @ yn + (b_out + w_out@b_g) + xq

Design notes:
 - ACT exp is the bottleneck engine at 1 elem/cycle/lane @1.2GHz: 42.5M
   exps/core => ~280us floor on ACT alone; the kernel runs at ~95% of that.
   A DVE/GPSIMD Schraudolph bit-trick exp (u16 = round(S*128*log2e + B)
   saturating at 0; the u16 bits ARE the bf16 exp; verified bit-exact on
   HW, end-to-end error ~1e-2 vs tolerance 2e-2) can offload a slice of
   the exp columns (BASS_NL_QA < 1024), but measured SLOWER: the ST-psum
   slot WAR chain exp(g) -> ST(g+2) -> exp(g+2) plus tile-granular WAR/WAW
   tracking and strict engine FIFOs serialize the rails; every offloaded
   element returns as ACT idle. Kept for reference, disabled by default.
 - The softmax denominator fold tree runs on [128,4096] P megatiles (4
   groups per tile): 3 tensor_tensor folds + 2 ones-matmuls per oct (vs 7
   [128,512] DVE ops + 1 matmul): fewer per-op overheads and less DVE time
   overall; fold steps are emitted only at phases where their inputs are
   already complete so the DVE FIFO head never blocks.
 - reciprocal_approx_fast (1 DVE op, ~51 ULP) replaces the 6.5 cyc/elem
   iterative reciprocal; yn = bf16(yT*rl) with an early yT copy that frees
   the Y accumulator bank before the fold flush at chunk boundaries.
 - phi's bias-add + f16 cast rides ACT's free affine (Identity, in the exp
   table set) to keep chunk-0 DVE load down; chunk 0 is PE-bound by the
   phi/g projections either way.
 - bf16/f16 moving operands stream through the PE at 1 col/cycle; scores
   are ACCUMULATED in fp32 PSUM (f16 inputs only perturb S by ~0.05 which
   the softmax normalization absorbs).
 - Emission is software-pipelined: each group's ST matmul is emitted BEFORE
   the previous group's Y matmuls (2-deep score lookahead) so the PE always
   has independent work while ACT computes exp; projections for the next
   chunk are hooked into fixed groups, the epilogue tail is deferred into
   the next chunk. A dummy exp at kernel start prefetches the ACT table set
   (~2.7us) off the critical path.

env BASS_NL_REPS=K wraps the whole computation in a K-iteration hardware
loop (idempotent recompute) for slope-based timing. Default 1.
env BASS_NL_QA=n sets the ACT/DVE exp split point (default 1024 = all-ACT;
measured fastest -- see the design note on why offload loses).
Startup DMAs are spread across the SP/ACT/GPSIMD-SWDGE issue queues to
shorten the ramp; the epilogue tail is emitted one channel-half per group.
"""

import os
from contextlib import ExitStack

import ml_dtypes
import numpy as np

import concourse.bass as bass
import concourse.mybir as mybir
import concourse.tile as tile
from concourse import bacc
from concourse.bass_utils import run_bass_kernel_spmd

F32 = mybir.dt.float32
BF16 = mybir.dt.bfloat16
F16 = mybir.dt.float16
U16 = mybir.dt.uint16
AF = mybir.ActivationFunctionType
ALU = mybir.AluOpType

C, N, D = 256, 9216, 128
NQ = N // 2            # queries per core
QCH = 512              # query chunk (one PSUM bank of fp32)
NQC = NQ // QCH        # 9 query chunks
MB = N // 128          # 72 key blocks
NG = MB // 2           # 36 key-block pairs (groups) per query chunk
NOC = NG // 4          # 9 octs per query chunk
NCH = N // 512         # 18 x-chunks (4 key blocks each)
SHIFT = -64.0          # softmax shift constant
LOG2E = 1.4426950408889634
SCH_A = 128.0 * LOG2E                       # Schraudolph scale
SCH_B = 128.0 * 127.0 + SHIFT * SCH_A - 5.6  # bias incl. minimax correction

_CACHE: dict = {}

def _build_nc():
    reps = int(os.environ.get("BASS_NL_REPS", "1"))
    # ACT/DVE exp split point (ACT computes score cols [0:qa), DVE-Schraudolph
    # the rest). 1024 = all-ACT: measured fastest — any offload loses more to
    # cross-engine WAR/WAW serialization of the ST-slot and P-tile than it
    # saves on ACT (tile-granular dep tracking + strict engine FIFOs).
    qa = int(os.environ.get("BASS_NL_QA", "1024"))
    nc = bacc.Bacc("TRN2", target_bir_lowering=False, debug=False, num_devices=8)
    xkb = nc.dram_tensor("xkb", [C, N], F16, kind="ExternalInput").ap()
    xqb = nc.dram_tensor("xqb", [C, NQ], F16, kind="ExternalInput").ap()
    xq = nc.dram_tensor("xq", [C, NQ], F32, kind="ExternalInput").ap()
    wth = nc.dram_tensor("wth", [C, D], F16, kind="ExternalInput").ap()
    wph = nc.dram_tensor("wph", [C, D], F16, kind="ExternalInput").ap()
    wg = nc.dram_tensor("wg", [C, D], F16, kind="ExternalInput").ap()
    wo = nc.dram_tensor("wo", [D, C], BF16, kind="ExternalInput").ap()
    bth = nc.dram_tensor("bth", [D, 1], F32, kind="ExternalInput").ap()
    bph = nc.dram_tensor("bph", [D, 1], F32, kind="ExternalInput").ap()
    bo2 = nc.dram_tensor("bo2", [C, 1], F32, kind="ExternalInput").ap()
    onesd = nc.dram_tensor("onesd", [D, D], BF16, kind="ExternalInput").ap()
    out = nc.dram_tensor("out", [C, NQ], F32, kind="ExternalOutput").ap()

    with tile.TileContext(nc) as tc, ExitStack() as ctx:
        consts = ctx.enter_context(tc.tile_pool(name="consts", bufs=1))
        big = ctx.enter_context(tc.tile_pool(name="big", bufs=1))

        # Persistent SBUF tensors, chunked for block-granular dependencies
        phi_t = [big.tile([128, 512], F16, name=f"phi{i}", tag=f"phi{i}") for i in range(NCH)]
        g_t = [big.tile([128, 512], BF16, name=f"g{i}", tag=f"g{i}") for i in range(NCH)]
        th_t = [big.tile([128, 512], F16, name=f"th{i}", tag=f"th{i}") for i in range(NQC)]

        wth_s = consts.tile([128, 256], F16)
        wph_s = consts.tile([128, 256], F16)
        wg_s = consts.tile([128, 256], F16)
        wo_s = consts.tile([128, 256], BF16)
        bth_s = consts.tile([128, 1], F32)
        bph_s = consts.tile([128, 1], F32)
        bo2_s = consts.tile([128, 2], F32)
        neg_s = consts.tile([128, 1], F32)
        scr_s = consts.tile([128, 1], F32)
        ones_s = consts.tile([128, 128], BF16)

        def body():
            # prefetch the exp table set while the first DMAs are in flight;
            # const DMAs are ordered so the first-phi-chunk critical chain
            # (wph, bph, xk chunk) issues ahead of everything else on the
            # HWDGE FIFO.
            nc.vector.memset(neg_s[:], SHIFT)
            nc.scalar.activation(scr_s[:], neg_s[:], AF.Exp)

            def late_consts():
                nc.sync.dma_start(out=ones_s[:], in_=onesd[:])
                nc.sync.dma_start(out=wo_s[:], in_=wo[:])
                nc.sync.dma_start(out=bo2_s[:, 0:1], in_=bo2[0:128, :])
                nc.sync.dma_start(out=bo2_s[:, 1:2], in_=bo2[128:256, :])

            with tc.tile_pool(name="psA", bufs=2, space="PSUM") as psA, tc.tile_pool(
                name="xsA", bufs=3
            ) as xsA, tc.tile_pool(name="st", bufs=2, space="PSUM") as stp, tc.tile_pool(
                name="yps", bufs=1, space="PSUM"
            ) as ypp, tc.tile_pool(
                name="lps", bufs=1, space="PSUM"
            ) as lpp, tc.tile_pool(
                name="pexp", bufs=3
            ) as pxp, tc.tile_pool(name="red", bufs=2) as red, tc.tile_pool(
                name="epi", bufs=3
            ) as epi:

                def emit_theta(i, first=False):
                    sl = bass.ts(i, 512)
                    # at startup, issue the theta-path DMAs from the idle
                    # GPSIMD SWDGE queue so they don't serialize behind the
                    # phi chain's DMAs on the SP queue (ramp-critical)
                    eng = nc.gpsimd if first else nc.sync
                    if first:
                        eng.dma_start(out=wth_s[:, 0:128], in_=wth[0:128, :])
                        eng.dma_start(
                            out=wth_s[:, 128:256], in_=wth[128:256, :]
                        )
                        eng.dma_start(out=bth_s[:], in_=bth[:])
                    xq0 = xsA.tile([128, 512], F16, tag="xq0")
                    xq1 = xsA.tile([128, 512], F16, tag="xq1")
                    eng.dma_start(out=xq0[:], in_=xqb[0:128, sl])
                    eng.dma_start(out=xq1[:], in_=xqb[128:256, sl])
                    ps = psA.tile([128, 512], F32, tag="ps")
                    nc.tensor.matmul(
                        ps[:], lhsT=wth_s[:, 0:128], rhs=xq0[:], start=True, stop=False
                    )
                    nc.tensor.matmul(
                        ps[:], lhsT=wth_s[:, 128:256], rhs=xq1[:], start=False, stop=True
                    )
                    nc.vector.tensor_scalar_add(th_t[i][:], ps[:], bth_s[:])

                def emit_phi(i):
                    # phi chunk i and g chunk i share one xk-chunk DMA; the
                    # phi part must precede ST(i*2) in the PE FIFO, the g
                    # part is emitted after it (emit_g) to not delay exp.
                    sl = bass.ts(i, 512)
                    xc0 = xsA.tile([128, 512], F16, tag="xc0")
                    xc1 = xsA.tile([128, 512], F16, tag="xc1")
                    nc.sync.dma_start(out=xc0[:], in_=xkb[0:128, sl])
                    nc.sync.dma_start(out=xc1[:], in_=xkb[128:256, sl])
                    if i == 0:
                        # weight DMAs issued after the ramp-critical first
                        # xk chunk on the SP queue
                        nc.sync.dma_start(out=wph_s[:, 0:128], in_=wph[0:128, :])
                        nc.sync.dma_start(
                            out=wph_s[:, 128:256], in_=wph[128:256, :]
                        )
                        nc.sync.dma_start(out=bph_s[:], in_=bph[:])
                    ps = psA.tile([128, 512], F32, tag="ps")
                    nc.tensor.matmul(
                        ps[:], lhsT=wph_s[:, 0:128], rhs=xc0[:], start=True, stop=False
                    )
                    nc.tensor.matmul(
                        ps[:], lhsT=wph_s[:, 128:256], rhs=xc1[:], start=False, stop=True
                    )
                    # bias-add + f16 cast via ACT's free affine (Identity is
                    # in the exp table set): keeps chunk-0 DVE load down.
                    nc.scalar.activation(
                        phi_t[i][:], ps[:], AF.Identity, bias=bph_s[:]
                    )
                    return xc0, xc1

                def emit_g(i, xc0, xc1, js=(0, 1, 2, 3), pg=None):
                    # each 128-col g matmul is shorter (53ns) than its
                    # LDWEIGHTS (107ns), so bursts of them are LDW-bound;
                    # emitted in two halves interleaved between the 512-col
                    # ST/Y matmuls, whose streams hide the weight loads.
                    if pg is None:
                        pg = psA.tile([128, 512], F32, tag="ps")
                    for j in js:
                        jsl = bass.ts(j, 128)
                        nc.tensor.matmul(
                            pg[:, jsl], lhsT=xc0[:, jsl], rhs=wg_s[:, 0:128],
                            start=True, stop=False,
                        )
                        nc.tensor.matmul(
                            pg[:, jsl], lhsT=xc1[:, jsl], rhs=wg_s[:, 128:256],
                            start=False, stop=True,
                        )
                    if js[-1] == 3:
                        nc.vector.tensor_copy(g_t[i][:], pg[:])
                    return pg

                def emit_st(qc, g):
                    # scores for key blocks 2g, 2g+1 vs query chunk qc.
                    # Block b1's matmul is split at the ACT/DVE boundary ka
                    # so the WAR chain exp(g) -> ST(g+2) -> exp(g+2) only
                    # contains the matmuls the exp actually reads; the
                    # Schraudolph columns [ka:1024) form their own (slack)
                    # DVE rail.
                    ka = 2 * QCH if qc == 0 else qa
                    st = stp.tile([128, 2 * QCH], F32, tag="st")
                    mb0, mb1 = 2 * g, 2 * g + 1
                    lhs0 = phi_t[mb0 // 4][:, bass.ts(mb0 % 4, 128)]
                    lhs1 = phi_t[mb1 // 4][:, bass.ts(mb1 % 4, 128)]
                    nc.tensor.matmul(
                        st[:, 0:QCH], lhsT=lhs0, rhs=th_t[qc][:],
                        start=True, stop=True,
                    )
                    if ka >= 2 * QCH:
                        nc.tensor.matmul(
                            st[:, QCH : 2 * QCH], lhsT=lhs1, rhs=th_t[qc][:],
                            start=True, stop=True,
                        )
                    else:
                        kb = ka - QCH
                        nc.tensor.matmul(
                            st[:, QCH:ka], lhsT=lhs1, rhs=th_t[qc][:, 0:kb],
                            start=True, stop=True,
                        )
                        nc.tensor.matmul(
                            st[:, ka : 2 * QCH], lhsT=lhs1,
                            rhs=th_t[qc][:, kb:QCH], start=True, stop=True,
                        )
                    return st

                def emit_epi_tail(eqsl, yn, xr0, chans=(0, 1)):
                    # z projection + residual + store for a finished query
                    # chunk; deferred into the NEXT chunk (one 128-channel
                    # half per group, so its PE matmuls never overflow one
                    # exp window and stall ACT).
                    for ch in chans:
                        csl = bass.ts(ch, 128)
                        z_ps = psA.tile([128, QCH], F32, tag="ps")
                        nc.tensor.matmul(
                            z_ps[:], lhsT=wo_s[:, csl],
                            rhs=yn[:], start=True, stop=True,
                        )
                        if ch == 0:
                            xr = xr0
                        else:
                            xr = epi.tile([128, QCH], F32, tag="xr1")
                            nc.sync.dma_start(out=xr[:], in_=xq[csl, eqsl])
                        zo = epi.tile([128, QCH], F32, tag=f"zo{ch}")
                        nc.vector.scalar_tensor_tensor(
                            zo[:], z_ps[:], bo2_s[:, ch : ch + 1], xr[:],
                            ALU.add, ALU.add,
                        )
                        nc.sync.dma_start(out=out[csl, eqsl], in_=zo[:])

                def emit_group(nqc, ng):
                    # auxiliary projection work is hooked one chunk AHEAD of
                    # the score tile that first needs it, covering the
                    # xk-DMA + matmul + bias latency of the phi chain
                    xcs = None
                    if nqc == 0 and ng % 2 == 0 and ng // 2 + 1 < NCH:
                        xcs = emit_phi(ng // 2 + 1)
                    st = emit_st(nqc, ng)
                    if xcs is not None:
                        pg = emit_g(ng // 2 + 1, *xcs, js=(0, 1))
                        g_stash.append((ng // 2 + 1, *xcs, (2, 3), pg))
                    return st

                M_of: dict = {}

                def get_M(gi):
                    # megatile holding P for the oct gi//4 (alloc on demand)
                    oct_i = gi // 4
                    if oct_i not in M_of:
                        M_of[oct_i] = pxp.tile(
                            [128, 8 * QCH], BF16, name="M", tag="M"
                        )
                    return M_of[oct_i]

                def emit_schr(st_cur, gi, ka):
                    # DVE Schraudolph bit-trick exp (u16 bits == bf16 exp)
                    # for score columns [ka:1024) -- ONE contiguous run
                    # (strided engine APs pay the fixed cost per run).
                    if ka >= 2 * QCH:
                        return
                    moff = (gi % 4) * 2 * QCH
                    M = get_M(gi)
                    nc.vector.tensor_scalar(
                        M[:, moff + ka : moff + 2 * QCH].bitcast(U16),
                        st_cur[:, ka : 2 * QCH], SCH_A, SCH_B,
                        ALU.mult, ALU.add,
                    )

                def emit_act_exp(st_cur, gi, ka):
                    # ACT spline exp for score columns [0:ka), contiguous
                    moff = (gi % 4) * 2 * QCH
                    M = get_M(gi)
                    nc.scalar.activation(
                        M[:, moff : moff + ka], st_cur[:, 0:ka],
                        AF.Exp, bias=neg_s[:],
                    )

                groups = [(qc, g) for qc in range(NQC) for g in range(NG)]
                pending = []
                g_stash = []
                pending_epi = None
                y_ps = l_ps = None
                f1 = f2 = oh = None
                fold_q = []

                def emit_fold(step, oct_i, cur_l, j):
                    # deferred fold steps for oct_i: q23 (next oct phase 0),
                    # oh + first ones-matmul (phase 1), second ones-matmul
                    # (phase 2). Inputs are complete at emission so the DVE
                    # FIFO never blocks, and the PE sees at most one 512-col
                    # extra matmul per exp window (the pair in one window
                    # overflowed the exp cadence and stalled ACT).
                    nonlocal f2, oh
                    if step == 0:
                        M = M_of[oct_i]
                        f2 = red.tile([128, 2 * QCH], BF16, name="f2", tag="f2")
                        nc.vector.tensor_add(
                            f2[:], M[:, 4 * QCH : 6 * QCH], M[:, 6 * QCH : 8 * QCH]
                        )
                        del M_of[oct_i]
                    elif step == 1:
                        oh = red.tile([128, 2 * QCH], BF16, name="oh", tag="oh")
                        nc.vector.tensor_add(oh[:], f1[:], f2[:])
                        nc.tensor.matmul(
                            cur_l[:], lhsT=ones_s[:], rhs=oh[:, 0:QCH],
                            start=(j == 0), stop=False,
                        )
                    else:
                        nc.tensor.matmul(
                            cur_l[:], lhsT=ones_s[:], rhs=oh[:, QCH : 2 * QCH],
                            start=False, stop=(j == NOC - 1),
                        )
                xcs = emit_phi(0)
                emit_theta(0, first=True)
                pending.append(emit_st(0, 0))
                nc.sync.dma_start(out=wg_s[:, 0:128], in_=wg[0:128, :])
                nc.sync.dma_start(out=wg_s[:, 128:256], in_=wg[128:256, :])
                emit_g(0, *xcs)
                xcs = emit_phi(1)
                emit_g(1, *xcs)
                late_consts()
                for idx, (qc, g) in enumerate(groups):
                    gi = qc * NG + g
                    qsl = bass.ts(qc, QCH)
                    if g == 0:
                        y_ps = ypp.tile([128, QCH], F32, tag="y")
                        l_ps = lpp.tile([128, QCH], F32, tag="l")
                    st_cur = pending.pop(0)
                    # chunk 0 is PE-bound by the phi/g projections: keep all
                    # exp on ACT there; later chunks offload [ka:1024) to DVE
                    ka = 2 * QCH if qc == 0 else qa
                    emit_act_exp(st_cur, gi, ka)
                    # Schraudolph inline: its ST input is 2 groups old so it
                    # never blocks the DVE FIFO head; it runs before any fold
                    # of this iteration, keeping the ST-slot WAR ACT-gated.
                    emit_schr(st_cur, gi, ka)
                    # persistent 2-deep score lookahead: the next TWO groups'
                    # ST matmuls always sit in the PE FIFO ahead of this
                    # group's Y matmuls, so nothing that waits on the DVE
                    # can ever starve ACT of its next exp input.
                    while len(pending) < 2 and idx + 1 + len(pending) < len(groups):
                        nqc, ng = groups[idx + 1 + len(pending)]
                        pending.append(emit_group(nqc, ng))
                    if pending_epi is not None and g in (2, 3):
                        emit_epi_tail(*pending_epi, chans=(g - 2,))
                        if g == 3:
                            pending_epi = None
                    M = get_M(gi)
                    moff = (g % 4) * 2 * QCH
                    for h in range(2):
                        mb = 2 * g + h
                        nc.tensor.matmul(
                            y_ps[:],
                            lhsT=g_t[mb // 4][:, bass.ts(mb % 4, 128)],
                            rhs=M[:, moff + h * QCH : moff + (h + 1) * QCH],
                            start=(mb == 0), stop=(mb == MB - 1),
                        )
                    if g_stash:
                        s = g_stash.pop(0)
                        emit_g(*s[:3], js=s[3], pg=s[4])
                    if qc < NQC - 1 and g == 20:
                        # theta for the NEXT query chunk, emitted after this
                        # group's Y matmuls so its PE work never sits between
                        # an exp and the ST pair that exp's successor needs
                        emit_theta(qc + 1)
                    # denominator folds: q01 inline at phase 2 (inputs are
                    # the phase-0/1 exps, already done); q23 and the oct
                    # finish deferred into the next oct's phases 0/1.
                    if fold_q:
                        emit_fold(*fold_q.pop(0))
                    if g % 4 == 2:
                        f1 = red.tile([128, 2 * QCH], BF16, name="f1", tag="f1")
                        nc.vector.tensor_add(
                            f1[:], M[:, 0 : 2 * QCH], M[:, 2 * QCH : 4 * QCH]
                        )
                    elif g % 4 == 3:
                        for step in range(3):
                            fold_q.append((step, gi // 4, l_ps, g // 4))
                    if g == NG - 1:
                        # free the Y accumulator bank first (bf16 copy), then
                        # flush the remaining folds, 1/L, and the normalized
                        # yn tile. The z/store tail is deferred into the next
                        # chunk. On the final chunk nothing reuses the Y bank,
                        # so normalize straight out of PSUM (shorter tail).
                        last = qc == NQC - 1
                        if not last:
                            yT = epi.tile([128, QCH], BF16, tag="yT")
                            nc.vector.tensor_copy(yT[:], y_ps[:])
                        while fold_q:
                            emit_fold(*fold_q.pop(0))
                        rl = epi.tile([128, QCH], F32, tag="rl")
                        nc.vector.reciprocal_approx_fast(rl[:], l_ps[:])
                        yn = epi.tile([128, QCH], BF16, tag="yn")
                        nc.vector.tensor_mul(
                            yn[:], y_ps[:] if last else yT[:], rl[:]
                        )
                        xr0 = epi.tile([128, QCH], F32, tag="xr0")
                        nc.sync.dma_start(out=xr0[:], in_=xq[0:128, qsl])
                        pending_epi = (qsl, yn, xr0)
                emit_epi_tail(*pending_epi)

        if reps > 1:
            with tc.For_i(0, reps, 1):
                body()
        else:
            body()

    nc.compile()
    return nc


def _get_nc():
    if "nc" not in _CACHE:
        _CACHE["nc"] = _build_nc()
    return _CACHE["nc"]


def _in_maps(x, w_theta, b_theta, w_phi, b_phi, w_g, b_g, w_out, b_out):
    bf = ml_dtypes.bfloat16
    f16 = np.float16
    bo2 = (b_out + w_out @ b_g).astype(np.float32).reshape(C, 1)
    shared = {
        "onesd": np.ones((D, D), bf),
        "wth": np.ascontiguousarray(w_theta.T).astype(f16),
        "wph": np.ascontiguousarray(w_phi.T).astype(f16),
        "wg": np.ascontiguousarray(w_g.T).astype(f16),
        "wo": np.ascontiguousarray(w_out.T).astype(bf),
        "bth": b_theta.reshape(D, 1).copy(),
        "bph": b_phi.reshape(D, 1).copy(),
        "bo2": bo2,
    }
    in_maps = []
    for core in range(8):
        b, qh = core // 2, core % 2
        xkc = np.ascontiguousarray(x[b].reshape(C, N))
        xqc = np.ascontiguousarray(xkc[:, qh * NQ : (qh + 1) * NQ])
        in_maps.append(
            {
                "xkb": xkc.astype(f16),
                "xqb": xqc.astype(f16),
                "xq": xqc,
                **shared,
            }
        )
    return in_maps


def kernel(x, w_theta, b_theta, w_phi, b_phi, w_g, b_g, w_out, b_out, **kw):
    x = np.asarray(x, np.float32)
    w_theta = np.asarray(w_theta, np.float32)
    b_theta = np.asarray(b_theta, np.float32)
    w_phi = np.asarray(w_phi, np.float32)
    b_phi = np.asarray(b_phi, np.float32)
    w_g = np.asarray(w_g, np.float32)
    b_g = np.asarray(b_g, np.float32)
    w_out = np.asarray(w_out, np.float32)
    b_out = np.asarray(b_out, np.float32)

    B = x.shape[0]
    nc = _get_nc()
    in_maps = _in_maps(
        x, w_theta, b_theta, w_phi, b_phi, w_g, b_g, w_out, b_out
    )

    res = run_bass_kernel_spmd(nc, in_maps, list(range(8)))
    z = np.empty((B, C, N), np.float32)
    for core in range(8):
        b, qh = core // 2, core % 2
        z[b][:, qh * NQ : (qh + 1) * NQ] = res.results[core]["out"]
    return z.reshape(x.shape)


# revision 41
# speedup vs baseline: 1.0149x; 1.0004x over previous
"""NonLocal block (B=4, C=256, H=W=96, D=128) on 8 TRN2 NeuronCores.

Sharding: 8 shards = (sample b = core//2) x (query half qh = core%2).
Each core handles 4608 queries vs all 9216 keys of its sample.

Per-core kernel (projection + attention matmuls in bf16/fp16 with fp32 PSUM
accumulation; scores in fp32, softmax split across ACT and DVE):
  thetaT [D, 4608]  = w_theta @ xq + b_theta          (f16)
  phiT   [D, 9216]  = w_phi   @ xk + b_phi            (f16)
  g      [9216, D]  = xk.T @ w_g.T                    (bf16; bias folded out)
  for each query chunk (512 q) and key-block pair g (2x128 keys):
    ST = phiT_blk.T @ thetaT_chunk          [128 keys, 2*512]  (PE, fp32 PSUM)
    P[:, queries 0:QA]   = exp(ST - 64)     bf16               (ACT spline exp)
    P[:, queries QA:512] = schraudolph(ST)  bf16-bits via u16  (DVE, 1 op)
    Y += g_blk.T @ P_half                   [128 d, 512 q]     (PE, psum accum)
  per oct (4 groups = 8 key blocks), P tiles live in one [128,4096] megatile:
    3-op DVE fold tree (q01+q23 -> oh, bf16 2x), one fold step per group
    L += ones128.T @ oh_half                (PE, two 512-col matmuls per oct,
                                             split across exp windows)
  rl = recip_approx(L); yn = bf16(yT*rl); z = wo.T @ yn + (b_out + w_out@b_g) + xq

Design notes:
 - ACT exp is the bottleneck engine at 1 elem/cycle/lane @1.2GHz: 42.5M
   exps/core => ~280us floor on ACT alone; the kernel runs at ~95% of that.
   A DVE/GPSIMD Schraudolph bit-trick exp (u16 = round(S*128*log2e + B)
   saturating at 0; the u16 bits ARE the bf16 exp; verified bit-exact on
   HW, end-to-end error ~1e-2 vs tolerance 2e-2) can offload a slice of
   the exp columns (BASS_NL_QA < 1024), but measured SLOWER: the ST-psum
   slot WAR chain exp(g) -> ST(g+2) -> exp(g+2) plus tile-granular WAR/WAW
   tracking and strict engine FIFOs serialize the rails; every offloaded
   element returns as ACT idle. Kept for reference, disabled by default.
 - The softmax denominator fold tree runs on [128,4096] P megatiles (4
   groups per tile): 3 tensor_tensor folds + 2 ones-matmuls per oct (vs 7
   [128,512] DVE ops + 1 matmul): fewer per-op overheads and less DVE time
   overall; fold steps are emitted only at phases where their inputs are
   already complete so the DVE FIFO head never blocks.
 - reciprocal_approx_fast (1 DVE op, ~51 ULP) replaces the 6.5 cyc/elem
   iterative reciprocal; yn = bf16(yT*rl) with an early yT copy that frees
   the Y accumulator bank before the fold flush at chunk boundaries.
 - phi's bias-add + f16 cast rides ACT's free affine (Identity, in the exp
   table set) to keep chunk-0 DVE load down; chunk 0 is PE-bound by the
   phi/g projections either way.
 - bf16/f16 moving operands stream through the PE at 1 col/cycle; scores
   are ACCUMULATED in fp32 PSUM (f16 inputs only perturb S by ~0.05 which
   the softmax normalization absorbs).
 - Emission is software-pipelined: each group's ST matmul is emitted BEFORE
   the previous group's Y matmuls (2-deep score lookahead) so the PE always
   has independent work while ACT computes exp; projections for the next
   chunk are hooked into fixed groups, the epilogue tail is deferred into
   the next chunk. A dummy exp at kernel start prefetches the ACT table set
   (~2.7us) off the critical path.

env BASS_NL_REPS=K wraps the whole computation in a K-iteration hardware
loop (idempotent recompute) for slope-based timing. Default 1.
env BASS_NL_QA=n sets the ACT/DVE exp split point (default 1024 = all-ACT;
measured fastest -- see the design note on why offload loses).
Startup DMAs are spread across the SP/ACT/GPSIMD-SWDGE issue queues to
shorten the ramp; the epilogue tail is emitted one channel-half per group.
"""

import os
from contextlib import ExitStack

import ml_dtypes
import numpy as np

import concourse.bass as bass
import concourse.mybir as mybir
import concourse.tile as tile
from concourse import bacc
from concourse.bass_utils import run_bass_kernel_spmd

F32 = mybir.dt.float32
BF16 = mybir.dt.bfloat16
F16 = mybir.dt.float16
U16 = mybir.dt.uint16
AF = mybir.ActivationFunctionType
ALU = mybir.AluOpType

C, N, D = 256, 9216, 128
NQ = N // 2            # queries per core
QCH = 512              # query chunk (one PSUM bank of fp32)
NQC = NQ // QCH        # 9 query chunks
MB = N // 128          # 72 key blocks
NG = MB // 2           # 36 key-block pairs (groups) per query chunk
NOC = NG // 4          # 9 octs per query chunk
NCH = N // 512         # 18 x-chunks (4 key blocks each)
SHIFT = -64.0          # softmax shift constant
LOG2E = 1.4426950408889634
SCH_A = 128.0 * LOG2E                       # Schraudolph scale
SCH_B = 128.0 * 127.0 + SHIFT * SCH_A - 5.6  # bias incl. minimax correction

_CACHE: dict = {}

def _build_nc():
    reps = int(os.environ.get("BASS_NL_REPS", "1"))
    # ACT/DVE exp split point (ACT computes score cols [0:qa), DVE-Schraudolph
    # the rest). 1024 = all-ACT: measured fastest — any offload loses more to
    # cross-engine WAR/WAW serialization of the ST-slot and P-tile than it
    # saves on ACT (tile-granular dep tracking + strict engine FIFOs).
    qa = int(os.environ.get("BASS_NL_QA", "1024"))
    nc = bacc.Bacc("TRN2", target_bir_lowering=False, debug=False, num_devices=8)
    xkb = nc.dram_tensor("xkb", [C, N], F16, kind="ExternalInput").ap()
    xqb = nc.dram_tensor("xqb", [C, NQ], F16, kind="ExternalInput").ap()
    xq = nc.dram_tensor("xq", [C, NQ], F32, kind="ExternalInput").ap()
    wth = nc.dram_tensor("wth", [C, D], F16, kind="ExternalInput").ap()
    wph = nc.dram_tensor("wph", [C, D], F16, kind="ExternalInput").ap()
    wg = nc.dram_tensor("wg", [C, D], F16, kind="ExternalInput").ap()
    wo = nc.dram_tensor("wo", [D, C], BF16, kind="ExternalInput").ap()
    bth = nc.dram_tensor("bth", [D, 1], F32, kind="ExternalInput").ap()
    bph = nc.dram_tensor("bph", [D, 1], F32, kind="ExternalInput").ap()
    bo2 = nc.dram_tensor("bo2", [C, 1], F32, kind="ExternalInput").ap()
    onesd = nc.dram_tensor("onesd", [D, D], BF16, kind="ExternalInput").ap()
    out = nc.dram_tensor("out", [C, NQ], F32, kind="ExternalOutput").ap()

    with tile.TileContext(nc) as tc, ExitStack() as ctx:
        consts = ctx.enter_context(tc.tile_pool(name="consts", bufs=1))
        big = ctx.enter_context(tc.tile_pool(name="big", bufs=1))

        # Persistent SBUF tensors, chunked for block-granular dependencies
        phi_t = [big.tile([128, 512], F16, name=f"phi{i}", tag=f"phi{i}") for i in range(NCH)]
        g_t = [big.tile([128, 512], BF16, name=f"g{i}", tag=f"g{i}") for i in range(NCH)]
        th_t = [big.tile([128, 512], F16, name=f"th{i}", tag=f"th{i}") for i in range(NQC)]

        wth_s = consts.tile([128, 256], F16)
        wph_s = consts.tile([128, 256], F16)
        wg_s = consts.tile([128, 256], F16)
        wo_s = consts.tile([128, 256], BF16)
        bth_s = consts.tile([128, 1], F32)
        bph_s = consts.tile([128, 1], F32)
        bo2_s = consts.tile([128, 2], F32)
        neg_s = consts.tile([128, 1], F32)
        scr_s = consts.tile([128, 1], F32)
        ones_s = consts.tile([128, 128], BF16)

        def body():
            # prefetch the exp table set while the first DMAs are in flight;
            # const DMAs are ordered so the first-phi-chunk critical chain
            # (wph, bph, xk chunk) issues ahead of everything else on the
            # HWDGE FIFO.
            nc.vector.memset(neg_s[:], SHIFT)
            nc.scalar.activation(scr_s[:], neg_s[:], AF.Exp)

            def late_consts():
                nc.sync.dma_start(out=ones_s[:], in_=onesd[:])
                nc.sync.dma_start(out=wo_s[:], in_=wo[:])
                nc.sync.dma_start(out=bo2_s[:, 0:1], in_=bo2[0:128, :])
                nc.sync.dma_start(out=bo2_s[:, 1:2], in_=bo2[128:256, :])

            with tc.tile_pool(name="psA", bufs=2, space="PSUM") as psA, tc.tile_pool(
                name="xsA", bufs=3
            ) as xsA, tc.tile_pool(name="st", bufs=2, space="PSUM") as stp, tc.tile_pool(
                name="yps", bufs=1, space="PSUM"
            ) as ypp, tc.tile_pool(
                name="lps", bufs=1, space="PSUM"
            ) as lpp, tc.tile_pool(
                name="pexp", bufs=3
            ) as pxp, tc.tile_pool(name="red", bufs=2) as red, tc.tile_pool(
                name="epi", bufs=3
            ) as epi:

                def emit_theta(i, first=False):
                    sl = bass.ts(i, 512)
                    # at startup, issue the theta-path DMAs from the idle
                    # GPSIMD SWDGE queue so they don't serialize behind the
                    # phi chain's DMAs on the SP queue (ramp-critical)
                    eng = nc.gpsimd if first else nc.sync
                    if first:
                        eng.dma_start(out=wth_s[:, 0:128], in_=wth[0:128, :])
                        eng.dma_start(
                            out=wth_s[:, 128:256], in_=wth[128:256, :]
                        )
                        eng.dma_start(out=bth_s[:], in_=bth[:])
                    xq0 = xsA.tile([128, 512], F16, tag="xq0")
                    xq1 = xsA.tile([128, 512], F16, tag="xq1")
                    eng.dma_start(out=xq0[:], in_=xqb[0:128, sl])
                    eng.dma_start(out=xq1[:], in_=xqb[128:256, sl])
                    ps = psA.tile([128, 512], F32, tag="ps")
                    nc.tensor.matmul(
                        ps[:], lhsT=wth_s[:, 0:128], rhs=xq0[:], start=True, stop=False
                    )
                    nc.tensor.matmul(
                        ps[:], lhsT=wth_s[:, 128:256], rhs=xq1[:], start=False, stop=True
                    )
                    if first:
                        # ramp-critical: ACT's free affine beats the DVE op
                        # (the DVE wait on the PE sem was on the exp(0) chain)
                        nc.scalar.activation(
                            th_t[i][:], ps[:], AF.Identity, bias=bth_s[:]
                        )
                    else:
                        nc.vector.tensor_scalar_add(th_t[i][:], ps[:], bth_s[:])

                def emit_phi(i):
                    # phi chunk i and g chunk i share one xk-chunk DMA; the
                    # phi part must precede ST(i*2) in the PE FIFO, the g
                    # part is emitted after it (emit_g) to not delay exp.
                    sl = bass.ts(i, 512)
                    xc0 = xsA.tile([128, 512], F16, tag="xc0")
                    xc1 = xsA.tile([128, 512], F16, tag="xc1")
                    nc.sync.dma_start(out=xc0[:], in_=xkb[0:128, sl])
                    nc.sync.dma_start(out=xc1[:], in_=xkb[128:256, sl])
                    if i == 0:
                        # weight DMAs issued after the ramp-critical first
                        # xk chunk on the SP queue
                        nc.sync.dma_start(out=wph_s[:, 0:128], in_=wph[0:128, :])
                        nc.sync.dma_start(
                            out=wph_s[:, 128:256], in_=wph[128:256, :]
                        )
                        nc.sync.dma_start(out=bph_s[:], in_=bph[:])
                    ps = psA.tile([128, 512], F32, tag="ps")
                    nc.tensor.matmul(
                        ps[:], lhsT=wph_s[:, 0:128], rhs=xc0[:], start=True, stop=False
                    )
                    nc.tensor.matmul(
                        ps[:], lhsT=wph_s[:, 128:256], rhs=xc1[:], start=False, stop=True
                    )
                    # bias-add + f16 cast via ACT's free affine (Identity is
                    # in the exp table set): keeps chunk-0 DVE load down.
                    nc.scalar.activation(
                        phi_t[i][:], ps[:], AF.Identity, bias=bph_s[:]
                    )
                    return xc0, xc1

                def emit_g(i, xc0, xc1, js=(0, 1, 2, 3), pg=None):
                    # each 128-col g matmul is shorter (53ns) than its
                    # LDWEIGHTS (107ns), so bursts of them are LDW-bound;
                    # emitted in two halves interleaved between the 512-col
                    # ST/Y matmuls, whose streams hide the weight loads.
                    if pg is None:
                        pg = psA.tile([128, 512], F32, tag="ps")
                    for j in js:
                        jsl = bass.ts(j, 128)
                        nc.tensor.matmul(
                            pg[:, jsl], lhsT=xc0[:, jsl], rhs=wg_s[:, 0:128],
                            start=True, stop=False,
                        )
                        nc.tensor.matmul(
                            pg[:, jsl], lhsT=xc1[:, jsl], rhs=wg_s[:, 128:256],
                            start=False, stop=True,
                        )
                    if js[-1] == 3:
                        nc.vector.tensor_copy(g_t[i][:], pg[:])
                    return pg

                def emit_st(qc, g):
                    # scores for key blocks 2g, 2g+1 vs query chunk qc.
                    # Block b1's matmul is split at the ACT/DVE boundary ka
                    # so the WAR chain exp(g) -> ST(g+2) -> exp(g+2) only
                    # contains the matmuls the exp actually reads; the
                    # Schraudolph columns [ka:1024) form their own (slack)
                    # DVE rail.
                    ka = 2 * QCH if qc == 0 else qa
                    st = stp.tile([128, 2 * QCH], F32, tag="st")
                    mb0, mb1 = 2 * g, 2 * g + 1
                    lhs0 = phi_t[mb0 // 4][:, bass.ts(mb0 % 4, 128)]
                    lhs1 = phi_t[mb1 // 4][:, bass.ts(mb1 % 4, 128)]
                    nc.tensor.matmul(
                        st[:, 0:QCH], lhsT=lhs0, rhs=th_t[qc][:],
                        start=True, stop=True,
                    )
                    if ka >= 2 * QCH:
                        nc.tensor.matmul(
                            st[:, QCH : 2 * QCH], lhsT=lhs1, rhs=th_t[qc][:],
                            start=True, stop=True,
                        )
                    else:
                        kb = ka - QCH
                        nc.tensor.matmul(
                            st[:, QCH:ka], lhsT=lhs1, rhs=th_t[qc][:, 0:kb],
                            start=True, stop=True,
                        )
                        nc.tensor.matmul(
                            st[:, ka : 2 * QCH], lhsT=lhs1,
                            rhs=th_t[qc][:, kb:QCH], start=True, stop=True,
                        )
                    return st

                def emit_epi_tail(eqsl, yn, xr0, chans=(0, 1)):
                    # z projection + residual + store for a finished query
                    # chunk; deferred into the NEXT chunk (one 128-channel
                    # half per group, so its PE matmuls never overflow one
                    # exp window and stall ACT).
                    for ch in chans:
                        csl = bass.ts(ch, 128)
                        z_ps = psA.tile([128, QCH], F32, tag="ps")
                        nc.tensor.matmul(
                            z_ps[:], lhsT=wo_s[:, csl],
                            rhs=yn[:], start=True, stop=True,
                        )
                        if ch == 0:
                            xr = xr0
                        else:
                            xr = epi.tile([128, QCH], F32, tag="xr1")
                            nc.sync.dma_start(out=xr[:], in_=xq[csl, eqsl])
                        zo = epi.tile([128, QCH], F32, tag=f"zo{ch}")
                        nc.vector.scalar_tensor_tensor(
                            zo[:], z_ps[:], bo2_s[:, ch : ch + 1], xr[:],
                            ALU.add, ALU.add,
                        )
                        nc.sync.dma_start(out=out[csl, eqsl], in_=zo[:])

                def emit_group(nqc, ng):
                    # auxiliary projection work is hooked one chunk AHEAD of
                    # the score tile that first needs it, covering the
                    # xk-DMA + matmul + bias latency of the phi chain
                    xcs = None
                    if nqc == 0 and ng % 2 == 0 and ng // 2 + 1 < NCH:
                        xcs = emit_phi(ng // 2 + 1)
                    st = emit_st(nqc, ng)
                    if xcs is not None:
                        pg = emit_g(ng // 2 + 1, *xcs, js=(0, 1))
                        g_stash.append((ng // 2 + 1, *xcs, (2, 3), pg))
                    return st

                M_of: dict = {}

                def get_M(gi):
                    # megatile holding P for the oct gi//4 (alloc on demand)
                    oct_i = gi // 4
                    if oct_i not in M_of:
                        M_of[oct_i] = pxp.tile(
                            [128, 8 * QCH], BF16, name="M", tag="M"
                        )
                    return M_of[oct_i]

                def emit_schr(st_cur, gi, ka):
                    # DVE Schraudolph bit-trick exp (u16 bits == bf16 exp)
                    # for score columns [ka:1024) -- ONE contiguous run
                    # (strided engine APs pay the fixed cost per run).
                    if ka >= 2 * QCH:
                        return
                    moff = (gi % 4) * 2 * QCH
                    M = get_M(gi)
                    nc.vector.tensor_scalar(
                        M[:, moff + ka : moff + 2 * QCH].bitcast(U16),
                        st_cur[:, ka : 2 * QCH], SCH_A, SCH_B,
                        ALU.mult, ALU.add,
                    )

                def emit_act_exp(st_cur, gi, ka):
                    # ACT spline exp for score columns [0:ka), contiguous
                    moff = (gi % 4) * 2 * QCH
                    M = get_M(gi)
                    nc.scalar.activation(
                        M[:, moff : moff + ka], st_cur[:, 0:ka],
                        AF.Exp, bias=neg_s[:],
                    )

                groups = [(qc, g) for qc in range(NQC) for g in range(NG)]
                pending = []
                g_stash = []
                pending_epi = None
                y_ps = l_ps = None
                f1 = f2 = oh = None
                fold_q = []

                def emit_fold(step, oct_i, cur_l, j):
                    # deferred fold steps for oct_i: q23 (next oct phase 0),
                    # oh + first ones-matmul (phase 1), second ones-matmul
                    # (phase 2). Inputs are complete at emission so the DVE
                    # FIFO never blocks, and the PE sees at most one 512-col
                    # extra matmul per exp window (the pair in one window
                    # overflowed the exp cadence and stalled ACT).
                    nonlocal f2, oh
                    if step == 0:
                        M = M_of[oct_i]
                        f2 = red.tile([128, 2 * QCH], BF16, name="f2", tag="f2")
                        nc.vector.tensor_add(
                            f2[:], M[:, 4 * QCH : 6 * QCH], M[:, 6 * QCH : 8 * QCH]
                        )
                        del M_of[oct_i]
                    elif step == 1:
                        oh = red.tile([128, 2 * QCH], BF16, name="oh", tag="oh")
                        nc.vector.tensor_add(oh[:], f1[:], f2[:])
                        nc.tensor.matmul(
                            cur_l[:], lhsT=ones_s[:], rhs=oh[:, 0:QCH],
                            start=(j == 0), stop=False,
                        )
                    else:
                        nc.tensor.matmul(
                            cur_l[:], lhsT=ones_s[:], rhs=oh[:, QCH : 2 * QCH],
                            start=False, stop=(j == NOC - 1),
                        )
                xcs = emit_phi(0)
                emit_theta(0, first=True)
                pending.append(emit_st(0, 0))
                nc.sync.dma_start(out=wg_s[:, 0:128], in_=wg[0:128, :])
                nc.sync.dma_start(out=wg_s[:, 128:256], in_=wg[128:256, :])
                emit_g(0, *xcs)
                xcs = emit_phi(1)
                emit_g(1, *xcs)
                late_consts()
                for idx, (qc, g) in enumerate(groups):
                    gi = qc * NG + g
                    qsl = bass.ts(qc, QCH)
                    if g == 0:
                        y_ps = ypp.tile([128, QCH], F32, tag="y")
                        l_ps = lpp.tile([128, QCH], F32, tag="l")
                    st_cur = pending.pop(0)
                    # chunk 0 is PE-bound by the phi/g projections: keep all
                    # exp on ACT there; later chunks offload [ka:1024) to DVE
                    ka = 2 * QCH if qc == 0 else qa
                    emit_act_exp(st_cur, gi, ka)
                    # Schraudolph inline: its ST input is 2 groups old so it
                    # never blocks the DVE FIFO head; it runs before any fold
                    # of this iteration, keeping the ST-slot WAR ACT-gated.
                    emit_schr(st_cur, gi, ka)
                    # persistent 2-deep score lookahead: the next TWO groups'
                    # ST matmuls always sit in the PE FIFO ahead of this
                    # group's Y matmuls, so nothing that waits on the DVE
                    # can ever starve ACT of its next exp input.
                    while len(pending) < 2 and idx + 1 + len(pending) < len(groups):
                        nqc, ng = groups[idx + 1 + len(pending)]
                        pending.append(emit_group(nqc, ng))
                    if pending_epi is not None and g in (2, 3):
                        emit_epi_tail(*pending_epi, chans=(g - 2,))
                        if g == 3:
                            pending_epi = None
                    M = get_M(gi)
                    moff = (g % 4) * 2 * QCH
                    for h in range(2):
                        mb = 2 * g + h
                        nc.tensor.matmul(
                            y_ps[:],
                            lhsT=g_t[mb // 4][:, bass.ts(mb % 4, 128)],
                            rhs=M[:, moff + h * QCH : moff + (h + 1) * QCH],
                            start=(mb == 0), stop=(mb == MB - 1),
                        )
                    if g_stash:
                        s = g_stash.pop(0)
                        emit_g(*s[:3], js=s[3], pg=s[4])
                    if qc < NQC - 1 and g == 20:
                        # theta for the NEXT query chunk, emitted after this
                        # group's Y matmuls so its PE work never sits between
                        # an exp and the ST pair that exp's successor needs
                        emit_theta(qc + 1)
                    # denominator folds: q01 inline at phase 2 (inputs are
                    # the phase-0/1 exps, already done); q23 and the oct
                    # finish deferred into the next oct's phases 0/1.
                    if fold_q:
                        emit_fold(*fold_q.pop(0))
                    if g % 4 == 2:
                        f1 = red.tile([128, 2 * QCH], BF16, name="f1", tag="f1")
                        nc.vector.tensor_add(
                            f1[:], M[:, 0 : 2 * QCH], M[:, 2 * QCH : 4 * QCH]
                        )
                    elif g % 4 == 3:
                        for step in range(3):
                            fold_q.append((step, gi // 4, l_ps, g // 4))
                    if g == NG - 1 and qc == NQC - 1:
                        # tail-critical: quarter 2 is already folded into t2
                        # before this (final) exp completes; only quarter 3
                        # remains on the serial chain afterwards.
                        t2 = red.tile([128, 2 * QCH], BF16, name="t2", tag="f2")
                        nc.vector.tensor_add(
                            t2[:], f1[:], M[:, 4 * QCH : 6 * QCH]
                        )
                        oh_l = red.tile([128, 2 * QCH], BF16, name="oh_l", tag="oh")
                        nc.vector.tensor_add(
                            oh_l[:], t2[:], M[:, 6 * QCH : 8 * QCH]
                        )
                        j = g // 4
                        nc.tensor.matmul(
                            l_ps[:], lhsT=ones_s[:], rhs=oh_l[:, 0:QCH],
                            start=(j == 0), stop=False,
                        )
                        nc.tensor.matmul(
                            l_ps[:], lhsT=ones_s[:], rhs=oh_l[:, QCH : 2 * QCH],
                            start=False, stop=(j == NOC - 1),
                        )
                        fold_q.clear()
                    if g == NG - 1:
                        # free the Y accumulator bank first (bf16 copy), then
                        # flush the remaining folds, 1/L, and the normalized
                        # yn tile. The z/store tail is deferred into the next
                        # chunk. On the final chunk nothing reuses the Y bank,
                        # so normalize straight out of PSUM (shorter tail).
                        last = qc == NQC - 1
                        if not last:
                            yT = epi.tile([128, QCH], BF16, tag="yT")
                            nc.vector.tensor_copy(yT[:], y_ps[:])
                        while fold_q:
                            emit_fold(*fold_q.pop(0))
                        rl = epi.tile([128, QCH], F32, tag="rl")
                        nc.vector.reciprocal_approx_fast(rl[:], l_ps[:])
                        yn = epi.tile([128, QCH], BF16, tag="yn")
                        nc.vector.tensor_mul(
                            yn[:], y_ps[:] if last else yT[:], rl[:]
                        )
                        xr0 = epi.tile([128, QCH], F32, tag="xr0")
                        nc.sync.dma_start(out=xr0[:], in_=xq[0:128, qsl])
                        pending_epi = (qsl, yn, xr0)
                emit_epi_tail(*pending_epi)

        if reps > 1:
            with tc.For_i(0, reps, 1):
                body()
        else:
            body()

    nc.compile()
    return nc


def _get_nc():
    if "nc" not in _CACHE:
        _CACHE["nc"] = _build_nc()
    return _CACHE["nc"]


def _in_maps(x, w_theta, b_theta, w_phi, b_phi, w_g, b_g, w_out, b_out):
    bf = ml_dtypes.bfloat16
    f16 = np.float16
    bo2 = (b_out + w_out @ b_g).astype(np.float32).reshape(C, 1)
    shared = {
        "onesd": np.ones((D, D), bf),
        "wth": np.ascontiguousarray(w_theta.T).astype(f16),
        "wph": np.ascontiguousarray(w_phi.T).astype(f16),
        "wg": np.ascontiguousarray(w_g.T).astype(f16),
        "wo": np.ascontiguousarray(w_out.T).astype(bf),
        "bth": b_theta.reshape(D, 1).copy(),
        "bph": b_phi.reshape(D, 1).copy(),
        "bo2": bo2,
    }
    in_maps = []
    for core in range(8):
        b, qh = core // 2, core % 2
        xkc = np.ascontiguousarray(x[b].reshape(C, N))
        xqc = np.ascontiguousarray(xkc[:, qh * NQ : (qh + 1) * NQ])
        in_maps.append(
            {
                "xkb": xkc.astype(f16),
                "xqb": xqc.astype(f16),
                "xq": xqc,
                **shared,
            }
        )
    return in_maps


def kernel(x, w_theta, b_theta, w_phi, b_phi, w_g, b_g, w_out, b_out, **kw):
    x = np.asarray(x, np.float32)
    w_theta = np.asarray(w_theta, np.float32)
    b_theta = np.asarray(b_theta, np.float32)
    w_phi = np.asarray(w_phi, np.float32)
    b_phi = np.asarray(b_phi, np.float32)
    w_g = np.asarray(w_g, np.float32)
    b_g = np.asarray(b_g, np.float32)
    w_out = np.asarray(w_out, np.float32)
    b_out = np.asarray(b_out, np.float32)

    B = x.shape[0]
    nc = _get_nc()
    in_maps = _in_maps(
        x, w_theta, b_theta, w_phi, b_phi, w_g, b_g, w_out, b_out
    )

    res = run_bass_kernel_spmd(nc, in_maps, list(range(8)))
    z = np.empty((B, C, N), np.float32)
    for core in range(8):
        b, qh = core // 2, core % 2
        z[b][:, qh * NQ : (qh + 1) * NQ] = res.results[core]["out"]
    return z.reshape(x.shape)
